# revision 19
# baseline (speedup 1.0000x reference)
"""Trainium2 Bass kernel for nn_CRITTransformer (ViT-style dense transformer).

kernel(**inputs) takes FULL inputs as in reference.setup_inputs() and returns
the FULL [8, 6, 128, 128] output. Data-parallel over batch across 8
NeuronCores (1 image per core), weights replicated.

Per-core layout (v3):
  - activations transposed, bf16 residual stream: h16[d=256 (2 tiles), s=1024]
  - attention per head-pair g: scores^T[k, q] per head j in a [128, 1024]
    PSUM tile (row-tiled K=32 matmuls, ring of 3 so the PE stays ahead of
    the ACT exp stream), Exp per head, one fused bias multiply per pair
    (exp(rpb) Toeplitz cache windows), then col-tiled M=64 PV matmuls
    (2 heads concurrent; softmax denominator rides as the 33rd lhsT
    column -> PSUM rows 32/96). Pass 2 of pair g-1 interleaves with
    pass 1 of pair g.
  - per-c z -> reciprocal -> PE broadcast -> oall scale, WO kt-ordered so
    it starts as soon as oall[0] is scaled
  - LayerNorm: stats via ones-column matmuls into [1, S] PSUM rows, row
    math full-S so Ln/Exp appear once per LN (fewer ACT table swaps)
  - PSUM map: A = 3x[128,1024] (scores ring | QK evac | V-proj | WO pair |
                                FFN fps pair | LN stats rows | embed | cls)
              C = 2x[128,512]  (PV accum | zrep | LN reps | FFN gps)
"""

import numpy as np

import concourse.bass as bass
import concourse.mybir as mybir
import concourse.tile as tile
from concourse import bacc
from concourse.bass_utils import run_bass_kernel_spmd

F32R = mybir.dt.float32r
F32 = mybir.dt.float32
BF16 = mybir.dt.bfloat16
AF = mybir.ActivationFunctionType
OP = mybir.AluOpType

B, C_IN, IMG, PP, D, NH, L, DFF, NCLS, MAXS = 8, 42, 128, 4, 256, 8, 4, 1024, 6, 1024
S = (IMG // PP) ** 2   # 1024
HD = D // NH           # 32
KIN = C_IN * PP * PP   # 672
KIN_PAD = 768
NKT = D // 128         # 2
NST = S // 128         # 8
VSTRIDE = NH * (HD + 1)  # 264 per s-tile in vall
VW = NST * VSTRIDE + 64  # 2176 (pad so M=64 PV lhsT slices stay in-bounds)
EPS = 1e-6
NPAIR = NH // 2        # 4 head pairs


def _build(nc, use_ln_affine, use_biases):
    def din(name, shape, dtype=F32R):
        return nc.dram_tensor(name, shape, dtype, kind="ExternalInput")

    x_unf = din("x_unf", [KIN_PAD, S])
    conv_w = din("conv_w", [KIN_PAD, D])
    pos_t = din("pos_t", [D, S], BF16)
    wq = din("wq", [L, D, D], BF16)
    wk = din("wk", [L, D, D], BF16)
    wv = din("wv", [L, D, D], BF16)
    wo = din("wo", [L, D, D], BF16)
    w1 = din("w1", [L, D, DFF], BF16)
    w2 = din("w2", [L, DFF, D], BF16)
    bcache = din("bcache", [L, NPAIR, 128, 2 * 1920], BF16)
    ones1 = din("ones1", [1, 128])
    oavgc = din("oavgc", [128, 1], BF16)
    sel4 = din("sel4", [4, 128], F32)
    vinit = din("vinit", [128, VW], BF16)
    cls_w = din("cls_w", [D, NCLS * PP * PP], BF16)
    if use_biases:
        bq = din("bq", [L, D, 1], F32)
        bk = din("bk", [L, D, 1], F32)
        bv = din("bv", [L, 128, D], F32)
        bo = din("bo", [L, D, 1], F32)
        b1 = din("b1", [L, DFF, 1], F32)
        b2 = din("b2", [L, D, 1], F32)
        convb = din("convb", [D, 1], F32)
        clsb = din("clsb", [NCLS * PP * PP, 1], F32)
    if use_ln_affine:
        ln1g = din("ln1g", [L, D, 1], F32)
        ln1b = din("ln1b", [L, D, 1], F32)
        ln2g = din("ln2g", [L, D, 1], F32)
        ln2b = din("ln2b", [L, D, 1], F32)
        lnfg = din("lnfg", [D, 1], F32)
        lnfb = din("lnfb", [D, 1], F32)

    out_pl = nc.dram_tensor("out_pl", [NCLS * PP * PP, S], F32,
                            kind="ExternalOutput")

    with tile.TileContext(nc) as tc:
        with (
            tc.tile_pool(name="res", bufs=1) as res,
            tc.tile_pool(name="io", bufs=3) as io,
            tc.tile_pool(name="wp", bufs=10) as wp,
            tc.tile_pool(name="w1p", bufs=3) as w1p,
            tc.tile_pool(name="w2p", bufs=9) as w2p,
            tc.tile_pool(name="bcp", bufs=3) as bcp,
            tc.tile_pool(name="ep", bufs=10) as ep,
            tc.tile_pool(name="sgp", bufs=4) as sgp,
            tc.tile_pool(name="rowp", bufs=8) as rowp,
            tc.tile_pool(name="msc", bufs=4) as msc,
            tc.tile_pool(name="gtp", bufs=4) as gtp,
            tc.tile_pool(name="pcl", bufs=4) as pcl,
            tc.tile_pool(name="psS", bufs=1, space="PSUM") as psS,  # 1x8KB
            tc.tile_pool(name="psA", bufs=1, space="PSUM") as psA,  # 1x4KB
            tc.tile_pool(name="psC", bufs=2, space="PSUM") as psC,  # 2x2KB
        ):
            # ---- constants ----
            ones1_t = res.tile([1, 128], F32R, tag="ones1")
            nc.sync.dma_start(ones1_t[:], ones1[:])
            oavgc_t = res.tile([128, 1], BF16, tag="oavgc")
            nc.sync.dma_start(oavgc_t[:], oavgc[:])
            sel4_t = res.tile([4, 128], F32, tag="sel4")
            nc.sync.dma_start(sel4_t[:], sel4[:])
            epst = res.tile([128, 1], F32, tag="eps")
            nc.vector.memset(epst[:], EPS)

            h16 = [res.tile([128, S], BF16, tag=f"h16{k}", name=f"h16_{k}")
                   for k in range(NKT)]
            xr = [res.tile([128, S], BF16, tag=f"xr{k}", name=f"xr{k}")
                  for k in range(NKT)]
            qt = [res.tile([128, S], BF16, tag=f"qt{c}", name=f"qt{c}")
                  for c in range(NKT)]
            ktsb = [res.tile([128, S], BF16, tag=f"kt{c}", name=f"ktsb{c}")
                    for c in range(NKT)]
            oall = [res.tile([128, S], BF16, tag=f"oall{c}", name=f"oall{c}")
                    for c in range(NKT)]
            vall = res.tile([128, VW], BF16, tag="vall")
            zall = res.tile([36, S], BF16, tag="zall")
            hf_t = [res.tile([128, S], BF16, tag=f"hf{i}", name=f"hf{i}")
                    for i in range(NKT)]
            nc.sync.dma_start(vall[:], vinit[:])

            def pcol(src_ap):
                t = pcl.tile([128, 1], F32, tag="pcol", name="pcol")
                nc.sync.dma_start(t[:src_ap.shape[0], :], src_ap)
                return t

            # ---- PE warm-up spin: keep HAM at K=8/8 from the start ----
            wub = res.tile([128, 512], BF16, tag="wub")
            nc.vector.memset(wub[:], 0.0)
            wups = psC.tile([128, 512], F32, tag="C", name="wups")
            for _ in range(48):
                nc.tensor.matmul(wups[:], wub[:, 0:128], wub[:],
                                 start=True, stop=True, skip_group_check=True)

            # ================= patch embedding =================
            embS = psS.tile([128, 2 * S], F32, tag="S", name="embS")
            for c in range(NKT):
                cps = embS[:, c * S:(c + 1) * S]
                for kt in range(6):
                    xt_ = io.tile([128, S], F32R, tag="io")
                    nc.sync.dma_start(xt_[:], x_unf[kt * 128:(kt + 1) * 128, :])
                    wt = wp.tile([128, 128], F32R, tag="wc")
                    nc.sync.dma_start(
                        wt[:], conv_w[kt * 128:(kt + 1) * 128,
                                      c * 128:(c + 1) * 128])
                    for hf in range(2):
                        nc.tensor.matmul(
                            embS[:, c * S + hf * 512:c * S + hf * 512 + 512],
                            wt[:], xt_[:, hf * 512:(hf + 1) * 512],
                            start=(kt == 0), stop=(kt == 5),
                            skip_group_check=True)
                post = io.tile([128, S], BF16, tag="io")
                nc.sync.dma_start(post[:], pos_t[c * 128:(c + 1) * 128, :])
                if use_biases:
                    u = msc.tile([128, S], F32, tag="sq", name="embu")
                    nc.vector.tensor_tensor(u[:], cps, post[:], OP.add)
                    nc.scalar.activation(h16[c][:], u[:], AF.Identity,
                                         bias=pcol(convb[c * 128:(c + 1) * 128, :])[:])
                else:
                    nc.vector.tensor_tensor(h16[c][:], cps, post[:], OP.add)

            # ================= layernorm helper =================
            def layernorm(xt, out_t, g_ap, b_ap):
                # squares (bf16, 2x rate)
                sq = []
                for kt in range(NKT):
                    t = msc.tile([128, S], BF16, tag="sq", name=f"lnsq{kt}")
                    nc.vector.tensor_tensor(t[:], xt[kt][:], xt[kt][:], OP.mult)
                    sq.append(t)
                srow = psS.tile([1, 2 * S], F32, tag="S", name="lnrows")
                mrow = srow[:, 0:S]
                qrow = srow[:, S:2 * S]
                for qh in range(2):
                    qsl = slice(qh * 512, qh * 512 + 512)
                    for kt in range(NKT):
                        nc.tensor.matmul(srow[:, qh * 512:qh * 512 + 512],
                                         oavgc_t[:], xt[kt][:, qsl],
                                         start=(kt == 0), stop=(kt == NKT - 1),
                                         skip_group_check=True)
                        nc.tensor.matmul(srow[:, S + qh * 512:S + qh * 512 + 512],
                                         oavgc_t[:], sq[kt][:, qsl],
                                         start=(kt == 0), stop=(kt == NKT - 1),
                                         skip_group_check=True)
                m2 = rowp.tile([1, S], F32, tag="row", name="m2")
                nc.scalar.activation(m2[:], mrow, AF.Square)
                mr = rowp.tile([1, S], F32, tag="row", name="mr")
                nc.vector.tensor_copy(mr[:], mrow)
                var = rowp.tile([1, S], F32, tag="row", name="var")
                nc.vector.tensor_tensor(var[:], qrow, m2[:], OP.subtract)
                rrow = rowp.tile([1, S], F32R, tag="row", name="rrow")
                nc.scalar.activation(rrow[:], var[:], AF.Ln, bias=epst[0:1, :])
                nc.scalar.activation(rrow[:], rrow[:], AF.Exp, scale=-0.5)
                arow = rowp.tile([1, S], F32R, tag="row", name="arow")
                nc.vector.scalar_tensor_tensor(arow[:], mr[:], -1.0,
                                               rrow[:], OP.mult, OP.mult)
                for qh in range(2):
                    qsl = slice(qh * 512, qh * 512 + 512)
                    rrep = psC.tile([128, 512], F32, tag="C", name="rrep")
                    nc.tensor.matmul(rrep[:], ones1_t[:], rrow[:, qsl],
                                     start=True, stop=True,
                                     skip_group_check=True)
                    arep = psC.tile([128, 512], F32, tag="C", name="arep")
                    nc.tensor.matmul(arep[:], ones1_t[:], arow[:, qsl],
                                     start=True, stop=True,
                                     skip_group_check=True)
                    for kt in range(NKT):
                        u = msc.tile([128, 512], F32, tag="sq", name="lnu")
                        nc.vector.tensor_tensor(u[:], xt[kt][:, qsl], rrep[:],
                                                OP.mult)
                        if g_ap is None:
                            nc.vector.tensor_tensor(out_t[kt][:, qsl], u[:],
                                                    arep[:], OP.add)
                        else:
                            u2 = msc.tile([128, 512], F32, tag="sq", name="lnu2")
                            nc.vector.tensor_tensor(u2[:], u[:], arep[:], OP.add)
                            nc.scalar.activation(out_t[kt][:, qsl], u2[:],
                                                 AF.Identity,
                                                 scale=pcol(g_ap[kt])[:],
                                                 bias=pcol(b_ap[kt])[:])

            # ================= transformer layers =================
            for l in range(L):
                # ---- Q^T, K^T ----
                qk_w = []
                for c in range(NKT):
                    wqt = wp.tile([128, D], BF16, tag="wqk", name=f"wq{c}")
                    wkt = wp.tile([128, D], BF16, tag="wqk", name=f"wk{c}")
                    for kt in range(NKT):
                        nc.sync.dma_start(
                            wqt[:, kt * 128:(kt + 1) * 128],
                            wq[l, kt * 128:(kt + 1) * 128,
                               c * 128:(c + 1) * 128])
                        nc.sync.dma_start(
                            wkt[:, kt * 128:(kt + 1) * 128],
                            wk[l, kt * 128:(kt + 1) * 128,
                               c * 128:(c + 1) * 128])
                    qk_w.append((wqt, wkt))

                wo_t = {}
                for c in range(NKT):
                    for kt in range(NKT):
                        wot = wp.tile([128, 128], BF16, tag="wc")
                        nc.sync.dma_start(
                            wot[:], wo[l, kt * 128:(kt + 1) * 128,
                                       c * 128:(c + 1) * 128])
                        wo_t[(c, kt)] = wot
                w1t = [w1p.tile([128, DFF], BF16, tag="w1", name=f"w1t{i}")
                       for i in range(NKT)]
                for kt in range(NKT):
                    nc.sync.dma_start(w1t[kt][:],
                                      w1[l, kt * 128:(kt + 1) * 128, :])
                w2t = [w2p.tile([128, D], BF16, tag="w2", name=f"w2t{i}")
                       for i in range(DFF // 128)]
                for kt in range(DFF // 128):
                    nc.sync.dma_start(w2t[kt][:],
                                      w2[l, kt * 128:(kt + 1) * 128, :])

                def emit_qk(c):
                    wqt, wkt = qk_w[c]
                    for qh in range(2):
                        qsl = slice(qh * 512, qh * 512 + 512)
                        qps = psC.tile([128, 512], F32, tag="C", name="qps")
                        kps = psC.tile([128, 512], F32, tag="C", name="kps")
                        for kt in range(NKT):
                            nc.tensor.matmul(qps[:],
                                             wqt[:, kt * 128:(kt + 1) * 128],
                                             h16[kt][:, qsl], start=(kt == 0),
                                             stop=(kt == NKT - 1),
                                             skip_group_check=True)
                            nc.tensor.matmul(kps[:],
                                             wkt[:, kt * 128:(kt + 1) * 128],
                                             h16[kt][:, qsl], start=(kt == 0),
                                             stop=(kt == NKT - 1),
                                             skip_group_check=True)
                        if use_biases:
                            nc.scalar.activation(
                                qt[c][:, qsl], qps[:], AF.Identity,
                                bias=pcol(bq[l, c * 128:(c + 1) * 128, :])[:])
                            nc.scalar.activation(
                                ktsb[c][:, qsl], kps[:], AF.Identity,
                                bias=pcol(bk[l, c * 128:(c + 1) * 128, :])[:])
                        else:
                            nc.scalar.copy(qt[c][:, qsl], qps[:])
                            nc.scalar.copy(ktsb[c][:, qsl], kps[:])

                emit_qk(0)

                # ---- V (s-partition layout, interleaved ones cols) ----
                wvt = [w1p.tile([128, D], BF16, tag="wv", name=f"wv{i}")
                       for i in range(NKT)]
                for kt in range(NKT):
                    nc.sync.dma_start(wvt[kt][:],
                                      wv[l, kt * 128:(kt + 1) * 128, :])
                if use_biases:
                    bvt = msc.tile([128, D], F32, tag="sq", name="bvt")
                    nc.sync.dma_start(bvt[:], bv[l])
                def emit_v(st):
                    vps = psC.tile([128, D], F32, tag="C", name="vps")
                    for kt in range(NKT):
                        nc.tensor.matmul(
                            vps[:], h16[kt][:, st * 128:(st + 1) * 128],
                            wvt[kt][:], start=(kt == 0),
                            stop=(kt == NKT - 1), skip_group_check=True)
                    base = st * VSTRIDE
                    dst = bass.AP(vall.tensor, vall[:].offset + base,
                                  [list(vall[:].ap[0]), [HD + 1, NH], [1, HD]])
                    if use_biases:
                        nc.vector.tensor_tensor(
                            dst, vps[:].rearrange("p (a b) -> p a b", a=NH),
                            bvt[:].rearrange("p (a b) -> p a b", a=NH), OP.add)
                    else:
                        nc.vector.tensor_copy(
                            dst, vps[:].rearrange("p (a b) -> p a b", a=NH))
                emit_qk(1)

                # ---- attention: software-pipelined pairs ----
                et_tiles = {}
                pv_tiles = {}
                bc_tiles = {}
                for g in range(NPAIR):
                    t = bcp.tile([128, 2 * 1920], BF16, tag="bc",
                                 name=f"bct{g}")
                    nc.sync.dma_start(t[:], bcache[l, g])
                    bc_tiles[g] = t

                sctL = psS.tile([128, 2048], F32, tag="S", name="sctL")

                def emit_pass1_kt(g, kt8):
                    chunk = g // 2
                    r0 = 64 * (g % 2)
                    et = ep.tile([128, 2048], BF16, tag="e", name=f"et{g}_{kt8}")
                    for j in range(2):
                        r = r0 + 32 * j
                        for qh in range(2):
                            nc.tensor.matmul(
                                sctL[:, j * 1024 + qh * 512:
                                     j * 1024 + qh * 512 + 512],
                                ktsb[chunk][r:r + 32,
                                            kt8 * 128:(kt8 + 1) * 128],
                                qt[chunk][r:r + 32,
                                          qh * 512:(qh + 1) * 512],
                                start=True, stop=True,
                                skip_group_check=True,
                                tile_position=(r, 0))
                        nc.scalar.activation(et[:, j * 1024:j * 1024 + 1024],
                                             sctL[:, j * 1024:j * 1024 + 1024],
                                             AF.Exp)
                    bct = bc_tiles[g]
                    bsrc = bass.AP(bct.tensor,
                                   bct[:].offset + (7 - kt8) * 128,
                                   [list(bct[:].ap[0]), [1920, 2], [1, 1024]])
                    nc.vector.tensor_tensor(
                        et[:].rearrange("p (a b) -> p a b", a=2),
                        et[:].rearrange("p (a b) -> p a b", a=2),
                        bsrc, OP.mult)
                    et_tiles[(g, kt8)] = et

                def emit_pass2_kt(g, kt8):
                    et = et_tiles.pop((g, kt8))
                    pv = pv_tiles[g]
                    base = kt8 * VSTRIDE
                    for qh in range(2):
                        for j in range(2):
                            h = 2 * g + j
                            nc.tensor.matmul(
                                pv[qh][64 * j:64 * j + 64, :],
                                vall[:, base + h * (HD + 1):
                                     base + h * (HD + 1) + 64],
                                et[:, j * 1024 + qh * 512:
                                   j * 1024 + qh * 512 + 512],
                                start=(kt8 == 0), stop=(kt8 == NST - 1),
                                skip_group_check=True,
                                tile_position=(0, 64 * j))

                def emit_pair_tail(g):
                    pv = pv_tiles.pop(g)
                    for qh in range(2):
                        stg = sgp.tile([128, 512], BF16, tag="stage",
                                       name="stg")
                        nc.vector.tensor_copy(stg[:], pv[qh][:])
                        c = g // 2
                        r = 64 * (g % 2)
                        qsl = slice(qh * 512, qh * 512 + 512)
                        nc.sync.dma_start(oall[c][r:r + 32, qsl], stg[0:32, :])
                        nc.sync.dma_start(oall[c][r + 32:r + 64, qsl],
                                          stg[64:96, :])
                        zr0 = 32 * (g // 2) + 2 * (g % 2)
                        nc.sync.dma_start(zall[zr0:zr0 + 1, qsl],
                                          stg[32:33, :])
                        nc.sync.dma_start(zall[zr0 + 1:zr0 + 2, qsl],
                                          stg[96:97, :])

                def emit_zscale(c):
                    zf = rowp.tile([4, S], F32, tag="row", name="zf")
                    nc.vector.tensor_copy(zf[:], zall[32 * c:32 * c + 4, :])
                    zr = rowp.tile([4, S], F32, tag="row", name="zr")
                    nc.vector.reciprocal_approx_fast(zr[:], zf[:])
                    for qh in range(2):
                        qsl = slice(qh * 512, qh * 512 + 512)
                        zrep = psC.tile([128, 512], F32, tag="C", name="zrep")
                        nc.tensor.matmul(zrep[:], sel4_t[:], zr[:, qsl],
                                         start=True, stop=True,
                                         skip_group_check=True)
                        nc.vector.tensor_tensor(oall[c][:, qsl],
                                                oall[c][:, qsl], zrep[:],
                                                OP.mult)

                def alloc_pv(g):
                    pv_tiles[g] = [psC.tile([128, 512], F32, tag="C",
                                            name=f"pv{g}_{qh}")
                                   for qh in range(2)]

                for g in range(NPAIR):
                    for kt8 in range(NST):
                        if g > 0:
                            if kt8 == 0:
                                alloc_pv(g - 1)
                            emit_pass2_kt(g - 1, kt8)
                        else:
                            emit_v(kt8)
                        emit_pass1_kt(g, kt8)
                    if g >= 1:
                        emit_pair_tail(g - 1)
                    if g == 2:
                        emit_zscale(0)
                alloc_pv(NPAIR - 1)
                for kt8 in range(NST):
                    emit_pass2_kt(NPAIR - 1, kt8)
                emit_pair_tail(NPAIR - 1)

                # ---- z-normalize + wo + residual ----
                apsA = psA.tile([128, S], F32, tag="A", name="wops0")
                apsS = psS.tile([128, 2 * S], F32, tag="S", name="wops1")
                aps = [apsA, apsS[:, 0:S]]
                for kt in range(NKT):
                    if kt == 1:
                        emit_zscale(1)
                    for c in range(NKT):
                        for hf in range(2):
                            dst = (apsA[:, hf * 512:(hf + 1) * 512] if c == 0
                                   else apsS[:, hf * 512:(hf + 1) * 512])
                            nc.tensor.matmul(
                                dst, wo_t[(c, kt)][:],
                                oall[kt][:, hf * 512:(hf + 1) * 512],
                                start=(kt == 0), stop=(kt == NKT - 1),
                                skip_group_check=True)
                for c in range(NKT):
                    if use_biases:
                        nc.vector.scalar_tensor_tensor(
                            xr[c][:], aps[c],
                            pcol(bo[l, c * 128:(c + 1) * 128, :])[:],
                            h16[c][:], OP.add, OP.add)
                    else:
                        nc.vector.tensor_tensor(xr[c][:], aps[c],
                                                h16[c][:], OP.add)
                if use_ln_affine:
                    layernorm(xr, h16,
                              [ln1g[l, k * 128:(k + 1) * 128, :] for k in range(NKT)],
                              [ln1b[l, k * 128:(k + 1) * 128, :] for k in range(NKT)])
                else:
                    layernorm(xr, h16, None, None)

                # ---- FFN ----
                fpsA = psA.tile([128, S], F32, tag="A", name="fps0")
                fpsS = psS.tile([128, 2 * S], F32, tag="S", name="fps1")
                fps = [fpsA, fpsS[:, 0:S]]
                for ch in range(DFF // 128):
                    b1c = (pcol(b1[l, ch * 128:(ch + 1) * 128, :])
                           if use_biases else None)
                    for qh in range(2):
                        qsl = slice(qh * 512, qh * 512 + 512)
                        gps = psC.tile([128, 512], F32, tag="C", name="gps")
                        for kt in range(NKT):
                            nc.tensor.matmul(
                                gps[:], w1t[kt][:, ch * 128:(ch + 1) * 128],
                                h16[kt][:, qsl], start=(kt == 0),
                                stop=(kt == NKT - 1), skip_group_check=True)
                        gt = gtp.tile([128, 512], BF16, tag="gt", name="gt")
                        if qh == 0:
                            nc.scalar.activation(
                                gt[:], gps[:], AF.Relu,
                                bias=(b1c[:] if b1c is not None else 0.0))
                        elif b1c is not None:
                            nc.vector.tensor_scalar(
                                gt[:], gps[:], b1c[:], 0.0, OP.add, OP.max)
                        else:
                            nc.vector.tensor_scalar_max(gt[:], gps[:], 0.0)
                        for c in range(NKT):
                            dst = (fpsA[:, qsl] if c == 0
                                   else fpsS[:, qsl])
                            nc.tensor.matmul(
                                dst, w2t[ch][:, c * 128:(c + 1) * 128],
                                gt[:], start=(ch == 0),
                                stop=(ch == DFF // 128 - 1),
                                skip_group_check=True)
                for c in range(NKT):
                    if use_biases:
                        nc.vector.scalar_tensor_tensor(
                            xr[c][:], fps[c],
                            pcol(b2[l, c * 128:(c + 1) * 128, :])[:],
                            h16[c][:], OP.add, OP.add)
                    else:
                        nc.vector.tensor_tensor(xr[c][:], fps[c],
                                                h16[c][:], OP.add)
                if use_ln_affine:
                    layernorm(xr, h16,
                              [ln2g[l, k * 128:(k + 1) * 128, :] for k in range(NKT)],
                              [ln2b[l, k * 128:(k + 1) * 128, :] for k in range(NKT)])
                else:
                    layernorm(xr, h16, None, None)

            # ================= final LN + classifier =================
            if use_ln_affine:
                layernorm(h16, hf_t,
                          [lnfg[k * 128:(k + 1) * 128, :] for k in range(NKT)],
                          [lnfb[k * 128:(k + 1) * 128, :] for k in range(NKT)])
            else:
                layernorm(h16, hf_t, None, None)
            cps = psA.tile([NCLS * PP * PP, S], F32, tag="A", name="clsps")
            for kt in range(NKT):
                cwt = wp.tile([128, NCLS * PP * PP], BF16, tag="wc")
                nc.sync.dma_start(cwt[:], cls_w[kt * 128:(kt + 1) * 128, :])
                for hf in range(2):
                    nc.tensor.matmul(cps[:, hf * 512:(hf + 1) * 512], cwt[:],
                                     hf_t[kt][:, hf * 512:(hf + 1) * 512],
                                     start=(kt == 0), stop=(kt == NKT - 1),
                                     skip_group_check=True)
            outt = io.tile([NCLS * PP * PP, S], F32, tag="io")
            if use_biases:
                nc.scalar.activation(outt[:], cps[:], AF.Identity,
                                     bias=pcol(clsb[:])[:])
            else:
                nc.scalar.copy(outt[:], cps[:])
            nc.sync.dma_start(out_pl[:], outt[:])


def _prep_host(inputs):
    f = lambda a: np.ascontiguousarray(np.asarray(a), dtype=np.float32)
    import ml_dtypes
    bf = lambda a: np.ascontiguousarray(a).astype(ml_dtypes.bfloat16)
    x = f(inputs["x"])
    conv_w = f(inputs["conv_w"])
    pos = f(inputs["pos_embed"])
    rpb = f(inputs["rpb"])

    xs = []
    for b in range(B):
        xb = x[b].reshape(C_IN, IMG // PP, PP, IMG // PP, PP)
        xb = xb.transpose(0, 2, 4, 1, 3).reshape(KIN, S)
        xp = np.zeros((KIN_PAD, S), np.float32)
        xp[:KIN] = xb
        xs.append(xp)

    w = {}
    cw = conv_w.reshape(D, C_IN, PP, PP).transpose(1, 2, 3, 0).reshape(KIN, D)
    cwp = np.zeros((KIN_PAD, D), np.float32)
    cwp[:KIN] = cw
    w["conv_w"] = cwp
    w["pos_t"] = bf(pos.reshape(S, D).T)
    scale = 1.0 / np.sqrt(np.float32(HD))
    w["wq"] = bf(np.transpose(f(inputs["wq"]), (0, 2, 1)) * scale)
    w["wk"] = bf(np.transpose(f(inputs["wk"]), (0, 2, 1)))
    w["wv"] = bf(np.transpose(f(inputs["wv"]), (0, 2, 1)))
    w["wo"] = bf(np.transpose(f(inputs["wo"]), (0, 2, 1)))
    w["w1"] = bf(np.transpose(f(inputs["w1"]), (0, 2, 1)))
    w["w2"] = bf(np.transpose(f(inputs["w2"]), (0, 2, 1)))
    bc = np.zeros((L, NH, 128, 1920), np.float32)
    for l in range(L):
        for hh in range(NH):
            th = np.ascontiguousarray(rpb[l, :, hh])
            bc[l, hh] = np.lib.stride_tricks.as_strided(
                th[127:], shape=(128, 1920), strides=(-4, 4))
    bc = np.exp(bc)
    bcp = bc.reshape(L, NPAIR, 2, 128, 1920).transpose(0, 1, 3, 2, 4)
    w["bcache"] = bf(bcp.reshape(L, NPAIR, 128, 2 * 1920))
    w["ones1"] = np.ones((1, 128), np.float32)
    w["oavgc"] = bf(np.full((128, 1), 1.0 / D, np.float32))
    sel4 = np.zeros((4, 128), np.float32)
    for p in range(128):
        sel4[p // 32, p] = 1.0
    w["sel4"] = sel4
    w["cls_w"] = bf(f(inputs["cls_w"]).T)
    vinit = np.zeros((128, VW), np.float32)
    for st in range(NST):
        for hh in range(NH):
            vinit[:, st * VSTRIDE + hh * (HD + 1) + HD] = 1.0
    w["vinit"] = bf(vinit)

    use_biases = any(
        np.abs(f(inputs[k])).max() > 0
        for k in ("bq", "bk", "bv", "bo", "b1", "b2", "conv_b", "cls_b"))
    use_ln_affine = not (
        np.allclose(f(inputs["ln1_s"]), 1.0)
        and np.allclose(f(inputs["ln2_s"]), 1.0)
        and np.allclose(f(inputs["lnf_s"]), 1.0)
        and np.abs(f(inputs["ln1_b"])).max() == 0
        and np.abs(f(inputs["ln2_b"])).max() == 0
        and np.abs(f(inputs["lnf_b"])).max() == 0)
    if use_biases:
        w["bq"] = f(inputs["bq"]).reshape(L, D, 1)
        w["bk"] = f(inputs["bk"]).reshape(L, D, 1)
        w["bv"] = np.ascontiguousarray(
            np.broadcast_to(f(inputs["bv"])[:, None, :], (L, 128, D)))
        w["bo"] = f(inputs["bo"]).reshape(L, D, 1)
        w["b1"] = f(inputs["b1"]).reshape(L, DFF, 1)
        w["b2"] = f(inputs["b2"]).reshape(L, D, 1)
        w["convb"] = f(inputs["conv_b"]).reshape(D, 1)
        w["clsb"] = f(inputs["cls_b"]).reshape(NCLS * PP * PP, 1)
    if use_ln_affine:
        w["ln1g"] = f(inputs["ln1_s"]).reshape(L, D, 1)
        w["ln1b"] = f(inputs["ln1_b"]).reshape(L, D, 1)
        w["ln2g"] = f(inputs["ln2_s"]).reshape(L, D, 1)
        w["ln2b"] = f(inputs["ln2_b"]).reshape(L, D, 1)
        w["lnfg"] = f(inputs["lnf_s"]).reshape(D, 1)
        w["lnfb"] = f(inputs["lnf_b"]).reshape(D, 1)
    return w, xs, use_ln_affine, use_biases


_RUN_KWARGS = {}


def kernel(**inputs):
    w, xs, use_ln_affine, use_biases = _prep_host(inputs)
    nc = bacc.Bacc("TRN2")
    _build(nc, use_ln_affine, use_biases)
    nc.finalize()
    in_maps = [dict(w, x_unf=xs[b]) for b in range(B)]
    res = run_bass_kernel_spmd(nc, in_maps, core_ids=list(range(B)),
                               **_RUN_KWARGS)
    kernel.last_result = res
    out = np.empty((B, NCLS, IMG, IMG), np.float32)
    for b in range(B):
        pl = res.results[b]["out_pl"]
        pl = pl.reshape(NCLS, PP, PP, IMG // PP, IMG // PP)
        out[b] = pl.transpose(0, 3, 1, 4, 2).reshape(NCLS, IMG, IMG)
    return out


# revision 20
# speedup vs baseline: 1.2113x; 1.2113x over previous
"""Trainium2 Bass kernel for nn_CRITTransformer (ViT-style dense transformer).

kernel(**inputs) takes FULL inputs as in reference.setup_inputs() and returns
the FULL [8, 6, 128, 128] output. Data-parallel over batch across 8
NeuronCores (1 image per core), weights replicated.

Per-core layout (v3):
  - activations transposed, bf16 residual stream: h16[d=256 (2 tiles), s=1024]
  - attention per head-pair g: scores^T[k, q] per head j in a [128, 1024]
    PSUM tile (row-tiled K=32 matmuls, ring of 3 so the PE stays ahead of
    the ACT exp stream), Exp per head, one fused bias multiply per pair
    (exp(rpb) Toeplitz cache windows), then col-tiled M=64 PV matmuls
    (2 heads concurrent; softmax denominator rides as the 33rd lhsT
    column -> PSUM rows 32/96). Pass 2 of pair g-1 interleaves with
    pass 1 of pair g.
  - per-c z -> reciprocal -> PE broadcast -> oall scale, WO kt-ordered so
    it starts as soon as oall[0] is scaled
  - LayerNorm: stats via ones-column matmuls into [1, S] PSUM rows, row
    math full-S so Ln/Exp appear once per LN (fewer ACT table swaps)
  - PSUM map: A = 3x[128,1024] (scores ring | QK evac | V-proj | WO pair |
                                FFN fps pair | LN stats rows | embed | cls)
              C = 2x[128,512]  (PV accum | zrep | LN reps | FFN gps)
"""

import numpy as np

import concourse.bass as bass
import concourse.mybir as mybir
import concourse.tile as tile
from concourse import bacc
from concourse.bass_utils import run_bass_kernel_spmd

F32R = mybir.dt.float32r
F32 = mybir.dt.float32
BF16 = mybir.dt.bfloat16
AF = mybir.ActivationFunctionType
OP = mybir.AluOpType

B, C_IN, IMG, PP, D, NH, L, DFF, NCLS, MAXS = 8, 42, 128, 4, 256, 8, 4, 1024, 6, 1024
S = (IMG // PP) ** 2   # 1024
HD = D // NH           # 32
KIN = C_IN * PP * PP   # 672
KIN_PAD = 768
NKT = D // 128         # 2
NST = S // 128         # 8
VSTRIDE = NH * (HD + 1)  # 264 per s-tile in vall
VW = NST * VSTRIDE + 64  # 2176 (pad so M=64 PV lhsT slices stay in-bounds)
EPS = 1e-6
NPAIR = NH // 2        # 4 head pairs


def _build(nc, use_ln_affine, use_biases):
    def din(name, shape, dtype=F32R):
        return nc.dram_tensor(name, shape, dtype, kind="ExternalInput")

    x_unf = din("x_unf", [KIN_PAD, S])
    conv_w = din("conv_w", [KIN_PAD, D])
    pos_t = din("pos_t", [D, S], BF16)
    wq = din("wq", [L, D, D], BF16)
    wk = din("wk", [L, D, D], BF16)
    wv = din("wv", [L, D, D], BF16)
    wo = din("wo", [L, D, D], BF16)
    w1 = din("w1", [L, D, DFF], BF16)
    w2 = din("w2", [L, DFF, D], BF16)
    bcache = din("bcache", [L, NPAIR, 128, 2 * 1920], BF16)
    ones1 = din("ones1", [1, 128])
    oavgc = din("oavgc", [128, 1], BF16)
    sel4 = din("sel4", [4, 128], F32)
    vinit = din("vinit", [128, VW], BF16)
    cls_w = din("cls_w", [D, NCLS * PP * PP], BF16)
    if use_biases:
        bq = din("bq", [L, D, 1], F32)
        bk = din("bk", [L, D, 1], F32)
        bv = din("bv", [L, 128, D], F32)
        bo = din("bo", [L, D, 1], F32)
        b1 = din("b1", [L, DFF, 1], F32)
        b2 = din("b2", [L, D, 1], F32)
        convb = din("convb", [D, 1], F32)
        clsb = din("clsb", [NCLS * PP * PP, 1], F32)
    if use_ln_affine:
        ln1g = din("ln1g", [L, D, 1], F32)
        ln1b = din("ln1b", [L, D, 1], F32)
        ln2g = din("ln2g", [L, D, 1], F32)
        ln2b = din("ln2b", [L, D, 1], F32)
        lnfg = din("lnfg", [D, 1], F32)
        lnfb = din("lnfb", [D, 1], F32)

    out_pl = nc.dram_tensor("out_pl", [NCLS * PP * PP, S], F32,
                            kind="ExternalOutput")

    with tile.TileContext(nc) as tc:
        with (
            tc.tile_pool(name="res", bufs=1) as res,
            tc.tile_pool(name="io", bufs=3) as io,
            tc.tile_pool(name="wp", bufs=10) as wp,
            tc.tile_pool(name="w1p", bufs=3) as w1p,
            tc.tile_pool(name="w2p", bufs=9) as w2p,
            tc.tile_pool(name="bcp", bufs=3) as bcp,
            tc.tile_pool(name="ep", bufs=10) as ep,
            tc.tile_pool(name="sgp", bufs=4) as sgp,
            tc.tile_pool(name="rowp", bufs=8) as rowp,
            tc.tile_pool(name="msc", bufs=4) as msc,
            tc.tile_pool(name="gtp", bufs=4) as gtp,
            tc.tile_pool(name="pcl", bufs=4) as pcl,
            tc.tile_pool(name="psA", bufs=3, space="PSUM") as psA,  # 3x4KB
            tc.tile_pool(name="psC", bufs=2, space="PSUM") as psC,  # 2x2KB
        ):
            # ---- constants ----
            ones1_t = res.tile([1, 128], F32R, tag="ones1")
            nc.sync.dma_start(ones1_t[:], ones1[:])
            oavgc_t = res.tile([128, 1], BF16, tag="oavgc")
            nc.sync.dma_start(oavgc_t[:], oavgc[:])
            sel4_t = res.tile([4, 128], F32, tag="sel4")
            nc.sync.dma_start(sel4_t[:], sel4[:])
            epst = res.tile([128, 1], F32, tag="eps")
            nc.vector.memset(epst[:], EPS)

            h16 = [res.tile([128, S], BF16, tag=f"h16{k}", name=f"h16_{k}")
                   for k in range(NKT)]
            xr = [res.tile([128, S], BF16, tag=f"xr{k}", name=f"xr{k}")
                  for k in range(NKT)]
            qt = [res.tile([128, S], BF16, tag=f"qt{c}", name=f"qt{c}")
                  for c in range(NKT)]
            ktsb = [res.tile([128, S], BF16, tag=f"kt{c}", name=f"ktsb{c}")
                    for c in range(NKT)]
            oall = [res.tile([128, S], BF16, tag=f"oall{c}", name=f"oall{c}")
                    for c in range(NKT)]
            vall = res.tile([128, VW], BF16, tag="vall")
            zall = res.tile([36, S], BF16, tag="zall")
            hf_t = [res.tile([128, S], BF16, tag=f"hf{i}", name=f"hf{i}")
                    for i in range(NKT)]
            nc.sync.dma_start(vall[:], vinit[:])

            def pcol(src_ap):
                t = pcl.tile([128, 1], F32, tag="pcol", name="pcol")
                nc.sync.dma_start(t[:src_ap.shape[0], :], src_ap)
                return t

            # ---- PE warm-up spin: keep HAM at K=8/8 from the start ----
            wub = res.tile([128, 512], BF16, tag="wub")
            nc.vector.memset(wub[:], 0.0)
            wups = psC.tile([128, 512], F32, tag="C", name="wups")
            for _ in range(48):
                nc.tensor.matmul(wups[:], wub[:, 0:128], wub[:],
                                 start=True, stop=True, skip_group_check=True)

            # ================= patch embedding =================
            for c in range(NKT):
                cps = psA.tile([128, S], F32, tag="A", name="emb")
                for kt in range(6):
                    xt_ = io.tile([128, S], F32R, tag="io")
                    nc.sync.dma_start(xt_[:], x_unf[kt * 128:(kt + 1) * 128, :])
                    wt = wp.tile([128, 128], F32R, tag="wc")
                    nc.sync.dma_start(
                        wt[:], conv_w[kt * 128:(kt + 1) * 128,
                                      c * 128:(c + 1) * 128])
                    for hf in range(2):
                        nc.tensor.matmul(
                            cps[:, hf * 512:(hf + 1) * 512],
                            wt[:], xt_[:, hf * 512:(hf + 1) * 512],
                            start=(kt == 0), stop=(kt == 5),
                            skip_group_check=True)
                post = io.tile([128, S], BF16, tag="io")
                nc.sync.dma_start(post[:], pos_t[c * 128:(c + 1) * 128, :])
                if use_biases:
                    u = msc.tile([128, S], F32, tag="sq", name="embu")
                    nc.vector.tensor_tensor(u[:], cps[:], post[:], OP.add)
                    nc.scalar.activation(h16[c][:], u[:], AF.Identity,
                                         bias=pcol(convb[c * 128:(c + 1) * 128, :])[:])
                else:
                    nc.vector.tensor_tensor(h16[c][:], cps[:], post[:], OP.add)

            # ================= layernorm helper =================
            def layernorm(xt, out_t, g_ap, b_ap):
                # squares (bf16, 2x rate)
                sq = []
                for kt in range(NKT):
                    t = msc.tile([128, S], BF16, tag="sq", name=f"lnsq{kt}")
                    nc.vector.tensor_tensor(t[:], xt[kt][:], xt[kt][:], OP.mult)
                    sq.append(t)
                mrow = psA.tile([1, S], F32, tag="A", name="mrow")
                qrow = psA.tile([1, S], F32, tag="A", name="qrow")
                for qh in range(2):
                    qsl = slice(qh * 512, qh * 512 + 512)
                    for kt in range(NKT):
                        nc.tensor.matmul(mrow[:, qsl], oavgc_t[:],
                                         xt[kt][:, qsl],
                                         start=(kt == 0), stop=(kt == NKT - 1),
                                         skip_group_check=True)
                        nc.tensor.matmul(qrow[:, qsl], oavgc_t[:],
                                         sq[kt][:, qsl],
                                         start=(kt == 0), stop=(kt == NKT - 1),
                                         skip_group_check=True)
                m2 = rowp.tile([1, S], F32, tag="row", name="m2")
                nc.scalar.activation(m2[:], mrow[:], AF.Square)
                mr = rowp.tile([1, S], F32, tag="row", name="mr")
                nc.vector.tensor_copy(mr[:], mrow[:])
                var = rowp.tile([1, S], F32, tag="row", name="var")
                nc.vector.tensor_tensor(var[:], qrow[:], m2[:], OP.subtract)
                rrow = rowp.tile([1, S], F32R, tag="row", name="rrow")
                nc.scalar.activation(rrow[:], var[:], AF.Ln, bias=epst[0:1, :])
                nc.scalar.activation(rrow[:], rrow[:], AF.Exp, scale=-0.5)
                arow = rowp.tile([1, S], F32R, tag="row", name="arow")
                nc.vector.scalar_tensor_tensor(arow[:], mr[:], -1.0,
                                               rrow[:], OP.mult, OP.mult)
                for qh in range(2):
                    qsl = slice(qh * 512, qh * 512 + 512)
                    rrep = psC.tile([128, 512], F32, tag="C", name="rrep")
                    nc.tensor.matmul(rrep[:], ones1_t[:], rrow[:, qsl],
                                     start=True, stop=True,
                                     skip_group_check=True)
                    arep = psC.tile([128, 512], F32, tag="C", name="arep")
                    nc.tensor.matmul(arep[:], ones1_t[:], arow[:, qsl],
                                     start=True, stop=True,
                                     skip_group_check=True)
                    for kt in range(NKT):
                        u = msc.tile([128, 512], F32, tag="sq", name="lnu")
                        nc.vector.tensor_tensor(u[:], xt[kt][:, qsl], rrep[:],
                                                OP.mult)
                        if g_ap is None:
                            nc.vector.tensor_tensor(out_t[kt][:, qsl], u[:],
                                                    arep[:], OP.add)
                        else:
                            u2 = msc.tile([128, 512], F32, tag="sq", name="lnu2")
                            nc.vector.tensor_tensor(u2[:], u[:], arep[:], OP.add)
                            nc.scalar.activation(out_t[kt][:, qsl], u2[:],
                                                 AF.Identity,
                                                 scale=pcol(g_ap[kt])[:],
                                                 bias=pcol(b_ap[kt])[:])

            # ================= transformer layers =================
            for l in range(L):
                # ---- Q^T, K^T ----
                qk_w = []
                for c in range(NKT):
                    wqt = wp.tile([128, D], BF16, tag="wqk", name=f"wq{c}")
                    wkt = wp.tile([128, D], BF16, tag="wqk", name=f"wk{c}")
                    for kt in range(NKT):
                        nc.sync.dma_start(
                            wqt[:, kt * 128:(kt + 1) * 128],
                            wq[l, kt * 128:(kt + 1) * 128,
                               c * 128:(c + 1) * 128])
                        nc.sync.dma_start(
                            wkt[:, kt * 128:(kt + 1) * 128],
                            wk[l, kt * 128:(kt + 1) * 128,
                               c * 128:(c + 1) * 128])
                    qk_w.append((wqt, wkt))

                wo_t = {}
                for c in range(NKT):
                    for kt in range(NKT):
                        wot = wp.tile([128, 128], BF16, tag="wc")
                        nc.sync.dma_start(
                            wot[:], wo[l, kt * 128:(kt + 1) * 128,
                                       c * 128:(c + 1) * 128])
                        wo_t[(c, kt)] = wot
                w1t = [w1p.tile([128, DFF], BF16, tag="w1", name=f"w1t{i}")
                       for i in range(NKT)]
                for kt in range(NKT):
                    nc.sync.dma_start(w1t[kt][:],
                                      w1[l, kt * 128:(kt + 1) * 128, :])
                w2t = [w2p.tile([128, D], BF16, tag="w2", name=f"w2t{i}")
                       for i in range(DFF // 128)]
                for kt in range(DFF // 128):
                    nc.sync.dma_start(w2t[kt][:],
                                      w2[l, kt * 128:(kt + 1) * 128, :])

                def emit_qk(c):
                    wqt, wkt = qk_w[c]
                    qps = psA.tile([128, S], F32, tag="A", name="qps")
                    kps = psA.tile([128, S], F32, tag="A", name="kps")
                    for qh in range(2):
                        qsl = slice(qh * 512, qh * 512 + 512)
                        for kt in range(NKT):
                            nc.tensor.matmul(qps[:, qsl],
                                             wqt[:, kt * 128:(kt + 1) * 128],
                                             h16[kt][:, qsl], start=(kt == 0),
                                             stop=(kt == NKT - 1),
                                             skip_group_check=True)
                            nc.tensor.matmul(kps[:, qsl],
                                             wkt[:, kt * 128:(kt + 1) * 128],
                                             h16[kt][:, qsl], start=(kt == 0),
                                             stop=(kt == NKT - 1),
                                             skip_group_check=True)
                    if use_biases:
                        nc.scalar.activation(
                            qt[c][:], qps[:], AF.Identity,
                            bias=pcol(bq[l, c * 128:(c + 1) * 128, :])[:])
                        nc.scalar.activation(
                            ktsb[c][:], kps[:], AF.Identity,
                            bias=pcol(bk[l, c * 128:(c + 1) * 128, :])[:])
                    else:
                        nc.scalar.copy(qt[c][:], qps[:])
                        nc.scalar.copy(ktsb[c][:], kps[:])

                emit_qk(0)

                # ---- V (s-partition layout, interleaved ones cols) ----
                wvt = [w1p.tile([128, D], BF16, tag="wv", name=f"wv{i}")
                       for i in range(NKT)]
                for kt in range(NKT):
                    nc.sync.dma_start(wvt[kt][:],
                                      wv[l, kt * 128:(kt + 1) * 128, :])
                if use_biases:
                    bvt = msc.tile([128, D], F32, tag="sq", name="bvt")
                    nc.sync.dma_start(bvt[:], bv[l])
                for st in range(NST):
                    vps = psA.tile([128, D], F32, tag="A", name="vps")
                    for kt in range(NKT):
                        nc.tensor.matmul(
                            vps[:], h16[kt][:, st * 128:(st + 1) * 128],
                            wvt[kt][:], start=(kt == 0),
                            stop=(kt == NKT - 1), skip_group_check=True)
                    base = st * VSTRIDE
                    dst = bass.AP(vall.tensor, vall[:].offset + base,
                                  [list(vall[:].ap[0]), [HD + 1, NH], [1, HD]])
                    if use_biases:
                        nc.vector.tensor_tensor(
                            dst, vps[:].rearrange("p (a b) -> p a b", a=NH),
                            bvt[:].rearrange("p (a b) -> p a b", a=NH), OP.add)
                    else:
                        nc.vector.tensor_copy(
                            dst, vps[:].rearrange("p (a b) -> p a b", a=NH))
                emit_qk(1)

                # ---- attention: software-pipelined pairs ----
                et_tiles = {}
                pv_tiles = {}
                bc_tiles = {}
                for g in range(NPAIR):
                    t = bcp.tile([128, 2 * 1920], BF16, tag="bc",
                                 name=f"bct{g}")
                    nc.sync.dma_start(t[:], bcache[l, g])
                    bc_tiles[g] = t

                def emit_pass1_kt(g, kt8):
                    chunk = g // 2
                    r0 = 64 * (g % 2)
                    et = ep.tile([128, 2048], BF16, tag="e", name=f"et{g}_{kt8}")
                    scts = [psA.tile([128, S], F32, tag="A", name="sct")
                            for _ in range(2)]
                    for j in range(2):
                        r = r0 + 32 * j
                        for qh in range(2):
                            nc.tensor.matmul(
                                scts[j][:, qh * 512:qh * 512 + 512],
                                ktsb[chunk][r:r + 32,
                                            kt8 * 128:(kt8 + 1) * 128],
                                qt[chunk][r:r + 32,
                                          qh * 512:(qh + 1) * 512],
                                start=True, stop=True,
                                skip_group_check=True,
                                tile_position=(r, 0))
                    for j in range(2):
                        nc.scalar.activation(et[:, j * 1024:j * 1024 + 1024],
                                             scts[j][:], AF.Exp)
                    bct = bc_tiles[g]
                    bsrc = bass.AP(bct.tensor,
                                   bct[:].offset + (7 - kt8) * 128,
                                   [list(bct[:].ap[0]), [1920, 2], [1, 1024]])
                    nc.vector.tensor_tensor(
                        et[:].rearrange("p (a b) -> p a b", a=2),
                        et[:].rearrange("p (a b) -> p a b", a=2),
                        bsrc, OP.mult)
                    et_tiles[(g, kt8)] = et

                def emit_pass2_kt(g, kt8):
                    et = et_tiles.pop((g, kt8))
                    pv = pv_tiles[g]
                    base = kt8 * VSTRIDE
                    for qh in range(2):
                        for j in range(2):
                            h = 2 * g + j
                            nc.tensor.matmul(
                                pv[qh][64 * j:64 * j + 64, :],
                                vall[:, base + h * (HD + 1):
                                     base + h * (HD + 1) + 64],
                                et[:, j * 1024 + qh * 512:
                                   j * 1024 + qh * 512 + 512],
                                start=(kt8 == 0), stop=(kt8 == NST - 1),
                                skip_group_check=True,
                                tile_position=(0, 64 * j))

                def emit_pair_tail(g):
                    pv = pv_tiles.pop(g)
                    for qh in range(2):
                        stg = sgp.tile([128, 512], BF16, tag="stage",
                                       name="stg")
                        nc.vector.tensor_copy(stg[:], pv[qh][:])
                        c = g // 2
                        r = 64 * (g % 2)
                        qsl = slice(qh * 512, qh * 512 + 512)
                        nc.sync.dma_start(oall[c][r:r + 32, qsl], stg[0:32, :])
                        nc.sync.dma_start(oall[c][r + 32:r + 64, qsl],
                                          stg[64:96, :])
                        zr0 = 32 * (g // 2) + 2 * (g % 2)
                        nc.sync.dma_start(zall[zr0:zr0 + 1, qsl],
                                          stg[32:33, :])
                        nc.sync.dma_start(zall[zr0 + 1:zr0 + 2, qsl],
                                          stg[96:97, :])

                def emit_zscale(c):
                    zf = rowp.tile([4, S], F32, tag="row", name="zf")
                    nc.vector.tensor_copy(zf[:], zall[32 * c:32 * c + 4, :])
                    zr = rowp.tile([4, S], F32, tag="row", name="zr")
                    nc.vector.reciprocal_approx_fast(zr[:], zf[:])
                    for qh in range(2):
                        qsl = slice(qh * 512, qh * 512 + 512)
                        zrep = psC.tile([128, 512], F32, tag="C", name="zrep")
                        nc.tensor.matmul(zrep[:], sel4_t[:], zr[:, qsl],
                                         start=True, stop=True,
                                         skip_group_check=True)
                        nc.vector.tensor_tensor(oall[c][:, qsl],
                                                oall[c][:, qsl], zrep[:],
                                                OP.mult)

                def alloc_pv(g):
                    pv_tiles[g] = [psC.tile([128, 512], F32, tag="C",
                                            name=f"pv{g}_{qh}")
                                   for qh in range(2)]

                for g in range(NPAIR):
                    for kt8 in range(NST):
                        if g > 0:
                            if kt8 == 0:
                                alloc_pv(g - 1)
                            emit_pass2_kt(g - 1, kt8)
                        emit_pass1_kt(g, kt8)
                    if g >= 1:
                        emit_pair_tail(g - 1)
                    if g == 2:
                        emit_zscale(0)
                alloc_pv(NPAIR - 1)
                for kt8 in range(NST):
                    emit_pass2_kt(NPAIR - 1, kt8)
                emit_pair_tail(NPAIR - 1)

                # ---- z-normalize + wo + residual ----
                aps = [psA.tile([128, S], F32, tag="A", name=f"wops{c}")
                       for c in range(NKT)]
                for kt in range(NKT):
                    if kt == 1:
                        emit_zscale(1)
                    for c in range(NKT):
                        for hf in range(2):
                            nc.tensor.matmul(
                                aps[c][:, hf * 512:(hf + 1) * 512],
                                wo_t[(c, kt)][:],
                                oall[kt][:, hf * 512:(hf + 1) * 512],
                                start=(kt == 0), stop=(kt == NKT - 1),
                                skip_group_check=True)
                for c in range(NKT):
                    if use_biases:
                        nc.vector.scalar_tensor_tensor(
                            xr[c][:], aps[c][:],
                            pcol(bo[l, c * 128:(c + 1) * 128, :])[:],
                            h16[c][:], OP.add, OP.add)
                    else:
                        nc.vector.tensor_tensor(xr[c][:], aps[c][:],
                                                h16[c][:], OP.add)
                if use_ln_affine:
                    layernorm(xr, h16,
                              [ln1g[l, k * 128:(k + 1) * 128, :] for k in range(NKT)],
                              [ln1b[l, k * 128:(k + 1) * 128, :] for k in range(NKT)])
                else:
                    layernorm(xr, h16, None, None)

                # ---- FFN ----
                fps = [psA.tile([128, S], F32, tag="A", name=f"fps{c}")
                       for c in range(NKT)]
                for ch in range(DFF // 128):
                    b1c = (pcol(b1[l, ch * 128:(ch + 1) * 128, :])
                           if use_biases else None)
                    for qh in range(2):
                        qsl = slice(qh * 512, qh * 512 + 512)
                        gps = psC.tile([128, 512], F32, tag="C", name="gps")
                        for kt in range(NKT):
                            nc.tensor.matmul(
                                gps[:], w1t[kt][:, ch * 128:(ch + 1) * 128],
                                h16[kt][:, qsl], start=(kt == 0),
                                stop=(kt == NKT - 1), skip_group_check=True)
                        gt = gtp.tile([128, 512], BF16, tag="gt", name="gt")
                        if qh == 0:
                            nc.scalar.activation(
                                gt[:], gps[:], AF.Relu,
                                bias=(b1c[:] if b1c is not None else 0.0))
                        elif b1c is not None:
                            nc.vector.tensor_scalar(
                                gt[:], gps[:], b1c[:], 0.0, OP.add, OP.max)
                        else:
                            nc.vector.tensor_scalar_max(gt[:], gps[:], 0.0)
                        for c in range(NKT):
                            nc.tensor.matmul(
                                fps[c][:, qsl],
                                w2t[ch][:, c * 128:(c + 1) * 128],
                                gt[:], start=(ch == 0),
                                stop=(ch == DFF // 128 - 1),
                                skip_group_check=True)
                for c in range(NKT):
                    if use_biases:
                        nc.vector.scalar_tensor_tensor(
                            xr[c][:], fps[c][:],
                            pcol(b2[l, c * 128:(c + 1) * 128, :])[:],
                            h16[c][:], OP.add, OP.add)
                    else:
                        nc.vector.tensor_tensor(xr[c][:], fps[c][:],
                                                h16[c][:], OP.add)
                if use_ln_affine:
                    layernorm(xr, h16,
                              [ln2g[l, k * 128:(k + 1) * 128, :] for k in range(NKT)],
                              [ln2b[l, k * 128:(k + 1) * 128, :] for k in range(NKT)])
                else:
                    layernorm(xr, h16, None, None)

            # ================= final LN + classifier =================
            if use_ln_affine:
                layernorm(h16, hf_t,
                          [lnfg[k * 128:(k + 1) * 128, :] for k in range(NKT)],
                          [lnfb[k * 128:(k + 1) * 128, :] for k in range(NKT)])
            else:
                layernorm(h16, hf_t, None, None)
            cps = psA.tile([NCLS * PP * PP, S], F32, tag="A", name="clsps")
            for kt in range(NKT):
                cwt = wp.tile([128, NCLS * PP * PP], BF16, tag="wc")
                nc.sync.dma_start(cwt[:], cls_w[kt * 128:(kt + 1) * 128, :])
                for hf in range(2):
                    nc.tensor.matmul(cps[:, hf * 512:(hf + 1) * 512], cwt[:],
                                     hf_t[kt][:, hf * 512:(hf + 1) * 512],
                                     start=(kt == 0), stop=(kt == NKT - 1),
                                     skip_group_check=True)
            outt = io.tile([NCLS * PP * PP, S], F32, tag="io")
            if use_biases:
                nc.scalar.activation(outt[:], cps[:], AF.Identity,
                                     bias=pcol(clsb[:])[:])
            else:
                nc.scalar.copy(outt[:], cps[:])
            nc.sync.dma_start(out_pl[:], outt[:])


def _prep_host(inputs):
    f = lambda a: np.ascontiguousarray(np.asarray(a), dtype=np.float32)
    import ml_dtypes
    bf = lambda a: np.ascontiguousarray(a).astype(ml_dtypes.bfloat16)
    x = f(inputs["x"])
    conv_w = f(inputs["conv_w"])
    pos = f(inputs["pos_embed"])
    rpb = f(inputs["rpb"])

    xs = []
    for b in range(B):
        xb = x[b].reshape(C_IN, IMG // PP, PP, IMG // PP, PP)
        xb = xb.transpose(0, 2, 4, 1, 3).reshape(KIN, S)
        xp = np.zeros((KIN_PAD, S), np.float32)
        xp[:KIN] = xb
        xs.append(xp)

    w = {}
    cw = conv_w.reshape(D, C_IN, PP, PP).transpose(1, 2, 3, 0).reshape(KIN, D)
    cwp = np.zeros((KIN_PAD, D), np.float32)
    cwp[:KIN] = cw
    w["conv_w"] = cwp
    w["pos_t"] = bf(pos.reshape(S, D).T)
    scale = 1.0 / np.sqrt(np.float32(HD))
    w["wq"] = bf(np.transpose(f(inputs["wq"]), (0, 2, 1)) * scale)
    w["wk"] = bf(np.transpose(f(inputs["wk"]), (0, 2, 1)))
    w["wv"] = bf(np.transpose(f(inputs["wv"]), (0, 2, 1)))
    w["wo"] = bf(np.transpose(f(inputs["wo"]), (0, 2, 1)))
    w["w1"] = bf(np.transpose(f(inputs["w1"]), (0, 2, 1)))
    w["w2"] = bf(np.transpose(f(inputs["w2"]), (0, 2, 1)))
    bc = np.zeros((L, NH, 128, 1920), np.float32)
    for l in range(L):
        for hh in range(NH):
            th = np.ascontiguousarray(rpb[l, :, hh])
            bc[l, hh] = np.lib.stride_tricks.as_strided(
                th[127:], shape=(128, 1920), strides=(-4, 4))
    bc = np.exp(bc)
    bcp = bc.reshape(L, NPAIR, 2, 128, 1920).transpose(0, 1, 3, 2, 4)
    w["bcache"] = bf(bcp.reshape(L, NPAIR, 128, 2 * 1920))
    w["ones1"] = np.ones((1, 128), np.float32)
    w["oavgc"] = bf(np.full((128, 1), 1.0 / D, np.float32))
    sel4 = np.zeros((4, 128), np.float32)
    for p in range(128):
        sel4[p // 32, p] = 1.0
    w["sel4"] = sel4
    w["cls_w"] = bf(f(inputs["cls_w"]).T)
    vinit = np.zeros((128, VW), np.float32)
    for st in range(NST):
        for hh in range(NH):
            vinit[:, st * VSTRIDE + hh * (HD + 1) + HD] = 1.0
    w["vinit"] = bf(vinit)

    use_biases = any(
        np.abs(f(inputs[k])).max() > 0
        for k in ("bq", "bk", "bv", "bo", "b1", "b2", "conv_b", "cls_b"))
    use_ln_affine = not (
        np.allclose(f(inputs["ln1_s"]), 1.0)
        and np.allclose(f(inputs["ln2_s"]), 1.0)
        and np.allclose(f(inputs["lnf_s"]), 1.0)
        and np.abs(f(inputs["ln1_b"])).max() == 0
        and np.abs(f(inputs["ln2_b"])).max() == 0
        and np.abs(f(inputs["lnf_b"])).max() == 0)
    if use_biases:
        w["bq"] = f(inputs["bq"]).reshape(L, D, 1)
        w["bk"] = f(inputs["bk"]).reshape(L, D, 1)
        w["bv"] = np.ascontiguousarray(
            np.broadcast_to(f(inputs["bv"])[:, None, :], (L, 128, D)))
        w["bo"] = f(inputs["bo"]).reshape(L, D, 1)
        w["b1"] = f(inputs["b1"]).reshape(L, DFF, 1)
        w["b2"] = f(inputs["b2"]).reshape(L, D, 1)
        w["convb"] = f(inputs["conv_b"]).reshape(D, 1)
        w["clsb"] = f(inputs["cls_b"]).reshape(NCLS * PP * PP, 1)
    if use_ln_affine:
        w["ln1g"] = f(inputs["ln1_s"]).reshape(L, D, 1)
        w["ln1b"] = f(inputs["ln1_b"]).reshape(L, D, 1)
        w["ln2g"] = f(inputs["ln2_s"]).reshape(L, D, 1)
        w["ln2b"] = f(inputs["ln2_b"]).reshape(L, D, 1)
        w["lnfg"] = f(inputs["lnf_s"]).reshape(D, 1)
        w["lnfb"] = f(inputs["lnf_b"]).reshape(D, 1)
    return w, xs, use_ln_affine, use_biases


_RUN_KWARGS = {}


def kernel(**inputs):
    w, xs, use_ln_affine, use_biases = _prep_host(inputs)
    nc = bacc.Bacc("TRN2")
    _build(nc, use_ln_affine, use_biases)
    nc.finalize()
    in_maps = [dict(w, x_unf=xs[b]) for b in range(B)]
    res = run_bass_kernel_spmd(nc, in_maps, core_ids=list(range(B)),
                               **_RUN_KWARGS)
    kernel.last_result = res
    out = np.empty((B, NCLS, IMG, IMG), np.float32)
    for b in range(B):
        pl = res.results[b]["out_pl"]
        pl = pl.reshape(NCLS, PP, PP, IMG // PP, IMG // PP)
        out[b] = pl.transpose(0, 3, 1, 4, 2).reshape(NCLS, IMG, IMG)
    return out


# revision 23
# speedup vs baseline: 1.2454x; 1.0282x over previous
"""Trainium2 Bass kernel for nn_CRITTransformer (ViT-style dense transformer).

kernel(**inputs) takes FULL inputs as in reference.setup_inputs() and returns
the FULL [8, 6, 128, 128] output. Data-parallel over batch across 8
NeuronCores (1 image per core), weights replicated.

Per-core layout (v3):
  - activations transposed, bf16 residual stream: h16[d=256 (2 tiles), s=1024]
  - attention per head-pair g: scores^T[k, q] per head j in a [128, 1024]
    PSUM tile (row-tiled K=32 matmuls, ring of 3 so the PE stays ahead of
    the ACT exp stream), Exp per head, one fused bias multiply per pair
    (exp(rpb) Toeplitz cache windows), then col-tiled M=64 PV matmuls
    (2 heads concurrent; softmax denominator rides as the 33rd lhsT
    column -> PSUM rows 32/96). Pass 2 of pair g-1 interleaves with
    pass 1 of pair g.
  - per-c z -> reciprocal -> PE broadcast -> oall scale, WO kt-ordered so
    it starts as soon as oall[0] is scaled
  - LayerNorm: stats via ones-column matmuls into [1, S] PSUM rows, row
    math full-S so Ln/Exp appear once per LN (fewer ACT table swaps)
  - PSUM map: A = 3x[128,1024] (scores ring | QK evac | V-proj | WO pair |
                                FFN fps pair | LN stats rows | embed | cls)
              C = 2x[128,512]  (PV accum | zrep | LN reps | FFN gps)
"""

import numpy as np

import concourse.bass as bass
import concourse.mybir as mybir
import concourse.tile as tile
from concourse import bacc
from concourse.bass_utils import run_bass_kernel_spmd

F32R = mybir.dt.float32r
F32 = mybir.dt.float32
BF16 = mybir.dt.bfloat16
AF = mybir.ActivationFunctionType
OP = mybir.AluOpType

B, C_IN, IMG, PP, D, NH, L, DFF, NCLS, MAXS = 8, 42, 128, 4, 256, 8, 4, 1024, 6, 1024
S = (IMG // PP) ** 2   # 1024
HD = D // NH           # 32
KIN = C_IN * PP * PP   # 672
KIN_PAD = 768
NKT = D // 128         # 2
NST = S // 128         # 8
VSTRIDE = NH * (HD + 1)  # 264 per s-tile in vall
VW = NST * VSTRIDE + 64  # 2176 (pad so M=64 PV lhsT slices stay in-bounds)
EPS = 1e-6
NPAIR = NH // 2        # 4 head pairs


def _build(nc, use_ln_affine, use_biases):
    def din(name, shape, dtype=F32R):
        return nc.dram_tensor(name, shape, dtype, kind="ExternalInput")

    x_unf = din("x_unf", [KIN_PAD, S])
    conv_w = din("conv_w", [KIN_PAD, D])
    pos_t = din("pos_t", [D, S], BF16)
    wq = din("wq", [L, D, D], BF16)
    wk = din("wk", [L, D, D], BF16)
    wv = din("wv", [L, D, D], BF16)
    wo = din("wo", [L, D, D], BF16)
    w1 = din("w1", [L, D, DFF], BF16)
    w2 = din("w2", [L, DFF, D], BF16)
    bcache = din("bcache", [L, NPAIR, 128, 2 * 1920], BF16)
    ones1 = din("ones1", [1, 128])
    oavgc = din("oavgc", [128, 1], BF16)
    sel4 = din("sel4", [4, 128], BF16)
    vinit = din("vinit", [128, VW], BF16)
    cls_w = din("cls_w", [D, NCLS * PP * PP], BF16)
    if use_biases:
        bq = din("bq", [L, D, 1], F32)
        bk = din("bk", [L, D, 1], F32)
        bv = din("bv", [L, 128, D], F32)
        bo = din("bo", [L, D, 1], F32)
        b1 = din("b1", [L, DFF, 1], F32)
        b2 = din("b2", [L, D, 1], F32)
        convb = din("convb", [D, 1], F32)
        clsb = din("clsb", [NCLS * PP * PP, 1], F32)
    if use_ln_affine:
        ln1g = din("ln1g", [L, D, 1], F32)
        ln1b = din("ln1b", [L, D, 1], F32)
        ln2g = din("ln2g", [L, D, 1], F32)
        ln2b = din("ln2b", [L, D, 1], F32)
        lnfg = din("lnfg", [D, 1], F32)
        lnfb = din("lnfb", [D, 1], F32)

    out_pl = nc.dram_tensor("out_pl", [NCLS * PP * PP, S], F32,
                            kind="ExternalOutput")

    with tile.TileContext(nc) as tc:
        with (
            tc.tile_pool(name="res", bufs=1) as res,
            tc.tile_pool(name="io", bufs=3) as io,
            tc.tile_pool(name="wp", bufs=10) as wp,
            tc.tile_pool(name="w1p", bufs=3) as w1p,
            tc.tile_pool(name="w2p", bufs=9) as w2p,
            tc.tile_pool(name="bcp", bufs=3) as bcp,
            tc.tile_pool(name="ep", bufs=10) as ep,
            tc.tile_pool(name="sgp", bufs=4) as sgp,
            tc.tile_pool(name="rowp", bufs=8) as rowp,
            tc.tile_pool(name="msc", bufs=4) as msc,
            tc.tile_pool(name="gtp", bufs=4) as gtp,
            tc.tile_pool(name="pcl", bufs=4) as pcl,
            tc.tile_pool(name="psA", bufs=3, space="PSUM") as psA,  # 3x4KB
            tc.tile_pool(name="psC", bufs=2, space="PSUM") as psC,  # 2x2KB
        ):
            # ---- constants ----
            ones1_t = res.tile([1, 128], F32R, tag="ones1")
            nc.sync.dma_start(ones1_t[:], ones1[:])
            oavgc_t = res.tile([128, 1], BF16, tag="oavgc")
            nc.sync.dma_start(oavgc_t[:], oavgc[:])
            sel4_t = res.tile([4, 128], BF16, tag="sel4")
            nc.sync.dma_start(sel4_t[:], sel4[:])
            epst = res.tile([128, 1], F32, tag="eps")
            nc.vector.memset(epst[:], EPS)

            h16 = [res.tile([128, S], BF16, tag=f"h16{k}", name=f"h16_{k}")
                   for k in range(NKT)]
            xr = [res.tile([128, S], BF16, tag=f"xr{k}", name=f"xr{k}")
                  for k in range(NKT)]
            qt = [res.tile([128, S], BF16, tag=f"qt{c}", name=f"qt{c}")
                  for c in range(NKT)]
            ktsb = [res.tile([128, S], BF16, tag=f"kt{c}", name=f"ktsb{c}")
                    for c in range(NKT)]
            oall = [res.tile([128, S], BF16, tag=f"oall{c}", name=f"oall{c}")
                    for c in range(NKT)]
            vall = res.tile([128, VW], BF16, tag="vall")
            zall = res.tile([36, S], BF16, tag="zall")
            hf_t = [res.tile([128, S], BF16, tag=f"hf{i}", name=f"hf{i}")
                    for i in range(NKT)]
            nc.sync.dma_start(vall[:], vinit[:])

            def pcol(src_ap):
                t = pcl.tile([128, 1], F32, tag="pcol", name="pcol")
                nc.sync.dma_start(t[:src_ap.shape[0], :], src_ap)
                return t

            # ---- PE warm-up spin: keep HAM at K=8/8 from the start ----
            wub = res.tile([128, 512], BF16, tag="wub")
            nc.vector.memset(wub[:], 0.0)
            wups = psC.tile([128, 512], F32, tag="C", name="wups")
            for _ in range(48):
                nc.tensor.matmul(wups[:], wub[:, 0:128], wub[:],
                                 start=True, stop=True, skip_group_check=True)

            # ================= patch embedding =================
            for c in range(NKT):
                cps = psA.tile([128, S], F32, tag="A", name="emb")
                for kt in range(6):
                    xt_ = io.tile([128, S], F32R, tag="io")
                    nc.sync.dma_start(xt_[:], x_unf[kt * 128:(kt + 1) * 128, :])
                    wt = wp.tile([128, 128], F32R, tag="wc")
                    nc.sync.dma_start(
                        wt[:], conv_w[kt * 128:(kt + 1) * 128,
                                      c * 128:(c + 1) * 128])
                    for hf in range(2):
                        nc.tensor.matmul(
                            cps[:, hf * 512:(hf + 1) * 512],
                            wt[:], xt_[:, hf * 512:(hf + 1) * 512],
                            start=(kt == 0), stop=(kt == 5),
                            skip_group_check=True)
                post = io.tile([128, S], BF16, tag="io")
                nc.sync.dma_start(post[:], pos_t[c * 128:(c + 1) * 128, :])
                if use_biases:
                    u = msc.tile([128, S], F32, tag="sq", name="embu")
                    nc.vector.tensor_tensor(u[:], cps[:], post[:], OP.add)
                    nc.scalar.activation(h16[c][:], u[:], AF.Identity,
                                         bias=pcol(convb[c * 128:(c + 1) * 128, :])[:])
                else:
                    nc.vector.tensor_tensor(h16[c][:], cps[:], post[:], OP.add)

            # ================= layernorm helper =================
            def layernorm(xt, out_t, g_ap, b_ap):
                # squares (bf16, 2x rate)
                sq = []
                for kt in range(NKT):
                    t = msc.tile([128, S], BF16, tag="sq", name=f"lnsq{kt}")
                    nc.vector.tensor_tensor(t[:], xt[kt][:], xt[kt][:], OP.mult)
                    sq.append(t)
                mrow = psA.tile([1, S], F32, tag="A", name="mrow")
                qrow = psA.tile([1, S], F32, tag="A", name="qrow")
                for qh in range(2):
                    qsl = slice(qh * 512, qh * 512 + 512)
                    for kt in range(NKT):
                        nc.tensor.matmul(mrow[:, qsl], oavgc_t[:],
                                         xt[kt][:, qsl],
                                         start=(kt == 0), stop=(kt == NKT - 1),
                                         skip_group_check=True)
                        nc.tensor.matmul(qrow[:, qsl], oavgc_t[:],
                                         sq[kt][:, qsl],
                                         start=(kt == 0), stop=(kt == NKT - 1),
                                         skip_group_check=True)
                m2 = rowp.tile([1, S], F32, tag="row", name="m2")
                nc.scalar.activation(m2[:], mrow[:], AF.Square)
                mr = rowp.tile([1, S], F32, tag="row", name="mr")
                nc.vector.tensor_copy(mr[:], mrow[:])
                var = rowp.tile([1, S], F32, tag="row", name="var")
                nc.vector.tensor_tensor(var[:], qrow[:], m2[:], OP.subtract)
                rrow = rowp.tile([1, S], F32R, tag="row", name="rrow")
                nc.scalar.activation(rrow[:], var[:], AF.Ln, bias=epst[0:1, :])
                nc.scalar.activation(rrow[:], rrow[:], AF.Exp, scale=-0.5)
                arow = rowp.tile([1, S], F32R, tag="row", name="arow")
                nc.vector.scalar_tensor_tensor(arow[:], mr[:], -1.0,
                                               rrow[:], OP.mult, OP.mult)
                for qh in range(2):
                    qsl = slice(qh * 512, qh * 512 + 512)
                    rrep = psC.tile([128, 512], F32, tag="C", name="rrep")
                    nc.tensor.matmul(rrep[:], ones1_t[:], rrow[:, qsl],
                                     start=True, stop=True,
                                     skip_group_check=True)
                    arep = psC.tile([128, 512], F32, tag="C", name="arep")
                    nc.tensor.matmul(arep[:], ones1_t[:], arow[:, qsl],
                                     start=True, stop=True,
                                     skip_group_check=True)
                    for kt in range(NKT):
                        u = msc.tile([128, 512], F32, tag="sq", name="lnu")
                        nc.vector.tensor_tensor(u[:], xt[kt][:, qsl], rrep[:],
                                                OP.mult)
                        if g_ap is None:
                            nc.vector.tensor_tensor(out_t[kt][:, qsl], u[:],
                                                    arep[:], OP.add)
                        else:
                            u2 = msc.tile([128, 512], F32, tag="sq", name="lnu2")
                            nc.vector.tensor_tensor(u2[:], u[:], arep[:], OP.add)
                            nc.scalar.activation(out_t[kt][:, qsl], u2[:],
                                                 AF.Identity,
                                                 scale=pcol(g_ap[kt])[:],
                                                 bias=pcol(b_ap[kt])[:])

            # ================= transformer layers =================
            for l in range(L):
                # ---- Q^T, K^T ----
                qk_w = []
                for c in range(NKT):
                    wqt = wp.tile([128, D], BF16, tag="wqk", name=f"wq{c}")
                    wkt = wp.tile([128, D], BF16, tag="wqk", name=f"wk{c}")
                    for kt in range(NKT):
                        nc.sync.dma_start(
                            wqt[:, kt * 128:(kt + 1) * 128],
                            wq[l, kt * 128:(kt + 1) * 128,
                               c * 128:(c + 1) * 128])
                        nc.sync.dma_start(
                            wkt[:, kt * 128:(kt + 1) * 128],
                            wk[l, kt * 128:(kt + 1) * 128,
                               c * 128:(c + 1) * 128])
                    qk_w.append((wqt, wkt))

                wo_t = {}
                for c in range(NKT):
                    for kt in range(NKT):
                        wot = wp.tile([128, 128], BF16, tag="wc")
                        nc.sync.dma_start(
                            wot[:], wo[l, kt * 128:(kt + 1) * 128,
                                       c * 128:(c + 1) * 128])
                        wo_t[(c, kt)] = wot
                w1t = [w1p.tile([128, DFF], BF16, tag="w1", name=f"w1t{i}")
                       for i in range(NKT)]
                for kt in range(NKT):
                    nc.sync.dma_start(w1t[kt][:],
                                      w1[l, kt * 128:(kt + 1) * 128, :])
                w2t = [w2p.tile([128, D], BF16, tag="w2", name=f"w2t{i}")
                       for i in range(DFF // 128)]
                for kt in range(DFF // 128):
                    nc.sync.dma_start(w2t[kt][:],
                                      w2[l, kt * 128:(kt + 1) * 128, :])

                def emit_qk(c):
                    wqt, wkt = qk_w[c]
                    qps = psA.tile([128, S], F32, tag="A", name="qps")
                    kps = psA.tile([128, S], F32, tag="A", name="kps")
                    for qh in range(2):
                        qsl = slice(qh * 512, qh * 512 + 512)
                        for kt in range(NKT):
                            nc.tensor.matmul(qps[:, qsl],
                                             wqt[:, kt * 128:(kt + 1) * 128],
                                             h16[kt][:, qsl], start=(kt == 0),
                                             stop=(kt == NKT - 1),
                                             skip_group_check=True)
                            nc.tensor.matmul(kps[:, qsl],
                                             wkt[:, kt * 128:(kt + 1) * 128],
                                             h16[kt][:, qsl], start=(kt == 0),
                                             stop=(kt == NKT - 1),
                                             skip_group_check=True)
                    if use_biases:
                        nc.scalar.activation(
                            qt[c][:], qps[:], AF.Identity,
                            bias=pcol(bq[l, c * 128:(c + 1) * 128, :])[:])
                        nc.scalar.activation(
                            ktsb[c][:], kps[:], AF.Identity,
                            bias=pcol(bk[l, c * 128:(c + 1) * 128, :])[:])
                    else:
                        nc.scalar.copy(qt[c][:], qps[:])
                        nc.scalar.copy(ktsb[c][:], kps[:])

                emit_qk(0)

                # ---- V (s-partition layout, interleaved ones cols) ----
                wvt = [w1p.tile([128, D], BF16, tag="wv", name=f"wv{i}")
                       for i in range(NKT)]
                for kt in range(NKT):
                    nc.sync.dma_start(wvt[kt][:],
                                      wv[l, kt * 128:(kt + 1) * 128, :])
                if use_biases:
                    bvt = msc.tile([128, D], F32, tag="sq", name="bvt")
                    nc.sync.dma_start(bvt[:], bv[l])
                for st in range(NST):
                    vps = psA.tile([128, D], F32, tag="A", name="vps")
                    for kt in range(NKT):
                        nc.tensor.matmul(
                            vps[:], h16[kt][:, st * 128:(st + 1) * 128],
                            wvt[kt][:], start=(kt == 0),
                            stop=(kt == NKT - 1), skip_group_check=True)
                    base = st * VSTRIDE
                    dst = bass.AP(vall.tensor, vall[:].offset + base,
                                  [list(vall[:].ap[0]), [HD + 1, NH], [1, HD]])
                    if use_biases:
                        nc.vector.tensor_tensor(
                            dst, vps[:].rearrange("p (a b) -> p a b", a=NH),
                            bvt[:].rearrange("p (a b) -> p a b", a=NH), OP.add)
                    else:
                        nc.vector.tensor_copy(
                            dst, vps[:].rearrange("p (a b) -> p a b", a=NH))
                emit_qk(1)

                # ---- attention: software-pipelined pairs ----
                et_tiles = {}
                pv_tiles = {}
                bc_tiles = {}
                for g in range(NPAIR):
                    t = bcp.tile([128, 2 * 1920], BF16, tag="bc",
                                 name=f"bct{g}")
                    nc.sync.dma_start(t[:], bcache[l, g])
                    bc_tiles[g] = t

                def emit_pass1_kt(g, kt8):
                    chunk = g // 2
                    r0 = 64 * (g % 2)
                    et = ep.tile([128, 2048], BF16, tag="e", name=f"et{g}_{kt8}")
                    scts = [psA.tile([128, S], F32, tag="A", name="sct")
                            for _ in range(2)]
                    for j in range(2):
                        r = r0 + 32 * j
                        for qh in range(2):
                            nc.tensor.matmul(
                                scts[j][:, qh * 512:qh * 512 + 512],
                                ktsb[chunk][r:r + 32,
                                            kt8 * 128:(kt8 + 1) * 128],
                                qt[chunk][r:r + 32,
                                          qh * 512:(qh + 1) * 512],
                                start=True, stop=True,
                                skip_group_check=True,
                                tile_position=(r, 0))
                    for j in range(2):
                        nc.scalar.activation(et[:, j * 1024:j * 1024 + 1024],
                                             scts[j][:], AF.Exp)
                    bct = bc_tiles[g]
                    bsrc = bass.AP(bct.tensor,
                                   bct[:].offset + (7 - kt8) * 128,
                                   [list(bct[:].ap[0]), [1920, 2], [1, 1024]])
                    nc.vector.tensor_tensor(
                        et[:].rearrange("p (a b) -> p a b", a=2),
                        et[:].rearrange("p (a b) -> p a b", a=2),
                        bsrc, OP.mult)
                    et_tiles[(g, kt8)] = et

                def emit_pass2_kt(g, kt8):
                    et = et_tiles.pop((g, kt8))
                    pv = pv_tiles[g]
                    base = kt8 * VSTRIDE
                    for qh in range(2):
                        for j in range(2):
                            h = 2 * g + j
                            nc.tensor.matmul(
                                pv[qh][64 * j:64 * j + 64, :],
                                vall[:, base + h * (HD + 1):
                                     base + h * (HD + 1) + 64],
                                et[:, j * 1024 + qh * 512:
                                   j * 1024 + qh * 512 + 512],
                                start=(kt8 == 0), stop=(kt8 == NST - 1),
                                skip_group_check=True,
                                tile_position=(0, 64 * j))

                def emit_pair_tail(g):
                    pv = pv_tiles.pop(g)
                    for qh in range(2):
                        stg = sgp.tile([128, 512], BF16, tag="stage",
                                       name="stg")
                        nc.vector.tensor_copy(stg[:], pv[qh][:])
                        c = g // 2
                        r = 64 * (g % 2)
                        qsl = slice(qh * 512, qh * 512 + 512)
                        nc.sync.dma_start(oall[c][r:r + 32, qsl], stg[0:32, :])
                        nc.sync.dma_start(oall[c][r + 32:r + 64, qsl],
                                          stg[64:96, :])
                        zr0 = 32 * (g // 2) + 2 * (g % 2)
                        nc.sync.dma_start(zall[zr0:zr0 + 1, qsl],
                                          stg[32:33, :])
                        nc.sync.dma_start(zall[zr0 + 1:zr0 + 2, qsl],
                                          stg[96:97, :])

                def emit_zscale(c):
                    zf = rowp.tile([4, S], F32, tag="row", name="zf")
                    nc.vector.tensor_copy(zf[:], zall[32 * c:32 * c + 4, :])
                    zr = rowp.tile([4, S], F32, tag="row", name="zr")
                    nc.vector.reciprocal_approx_fast(zr[:], zf[:])
                    zrc = rowp.tile([4, S], BF16, tag="row", name="zrc")
                    nc.vector.tensor_copy(zrc[:], zr[:])
                    for qh in range(2):
                        qsl = slice(qh * 512, qh * 512 + 512)
                        zrep = psC.tile([128, 512], F32, tag="C", name="zrep")
                        nc.tensor.matmul(zrep[:], sel4_t[:], zrc[:, qsl],
                                         start=True, stop=True,
                                         skip_group_check=True)
                        nc.vector.tensor_tensor(oall[c][:, qsl],
                                                oall[c][:, qsl], zrep[:],
                                                OP.mult)

                def alloc_pv(g):
                    pv_tiles[g] = [psC.tile([128, 512], F32, tag="C",
                                            name=f"pv{g}_{qh}")
                                   for qh in range(2)]

                for g in range(NPAIR):
                    for kt8 in range(NST):
                        if g > 0:
                            if kt8 == 0:
                                alloc_pv(g - 1)
                            emit_pass2_kt(g - 1, kt8)
                        emit_pass1_kt(g, kt8)
                    if g >= 1:
                        emit_pair_tail(g - 1)
                    if g == 2:
                        emit_zscale(0)
                alloc_pv(NPAIR - 1)
                for kt8 in range(NST):
                    emit_pass2_kt(NPAIR - 1, kt8)
                emit_pair_tail(NPAIR - 1)

                # ---- z-normalize + wo + residual ----
                aps = [psA.tile([128, S], F32, tag="A", name=f"wops{c}")
                       for c in range(NKT)]
                for kt in range(NKT):
                    if kt == 1:
                        emit_zscale(1)
                    for c in range(NKT):
                        for hf in range(2):
                            nc.tensor.matmul(
                                aps[c][:, hf * 512:(hf + 1) * 512],
                                wo_t[(c, kt)][:],
                                oall[kt][:, hf * 512:(hf + 1) * 512],
                                start=(kt == 0), stop=(kt == NKT - 1),
                                skip_group_check=True)
                for c in range(NKT):
                    if use_biases:
                        nc.vector.scalar_tensor_tensor(
                            xr[c][:], aps[c][:],
                            pcol(bo[l, c * 128:(c + 1) * 128, :])[:],
                            h16[c][:], OP.add, OP.add)
                    else:
                        nc.vector.tensor_tensor(xr[c][:], aps[c][:],
                                                h16[c][:], OP.add)
                if use_ln_affine:
                    layernorm(xr, h16,
                              [ln1g[l, k * 128:(k + 1) * 128, :] for k in range(NKT)],
                              [ln1b[l, k * 128:(k + 1) * 128, :] for k in range(NKT)])
                else:
                    layernorm(xr, h16, None, None)

                # ---- FFN ----
                fps = [psA.tile([128, S], F32, tag="A", name=f"fps{c}")
                       for c in range(NKT)]
                for ch in range(DFF // 128):
                    b1c = (pcol(b1[l, ch * 128:(ch + 1) * 128, :])
                           if use_biases else None)
                    for qh in range(2):
                        qsl = slice(qh * 512, qh * 512 + 512)
                        gps = psC.tile([128, 512], F32, tag="C", name="gps")
                        for kt in range(NKT):
                            nc.tensor.matmul(
                                gps[:], w1t[kt][:, ch * 128:(ch + 1) * 128],
                                h16[kt][:, qsl], start=(kt == 0),
                                stop=(kt == NKT - 1), skip_group_check=True)
                        gt = gtp.tile([128, 512], BF16, tag="gt", name="gt")
                        if qh == 0:
                            nc.scalar.activation(
                                gt[:], gps[:], AF.Relu,
                                bias=(b1c[:] if b1c is not None else 0.0))
                        elif b1c is not None:
                            nc.vector.tensor_scalar(
                                gt[:], gps[:], b1c[:], 0.0, OP.add, OP.max)
                        else:
                            nc.vector.tensor_scalar_max(gt[:], gps[:], 0.0)
                        for c in range(NKT):
                            nc.tensor.matmul(
                                fps[c][:, qsl],
                                w2t[ch][:, c * 128:(c + 1) * 128],
                                gt[:], start=(ch == 0),
                                stop=(ch == DFF // 128 - 1),
                                skip_group_check=True)
                for c in range(NKT):
                    if use_biases:
                        nc.vector.scalar_tensor_tensor(
                            xr[c][:], fps[c][:],
                            pcol(b2[l, c * 128:(c + 1) * 128, :])[:],
                            h16[c][:], OP.add, OP.add)
                    else:
                        nc.vector.tensor_tensor(xr[c][:], fps[c][:],
                                                h16[c][:], OP.add)
                if use_ln_affine:
                    layernorm(xr, h16,
                              [ln2g[l, k * 128:(k + 1) * 128, :] for k in range(NKT)],
                              [ln2b[l, k * 128:(k + 1) * 128, :] for k in range(NKT)])
                else:
                    layernorm(xr, h16, None, None)

            # ================= final LN + classifier =================
            if use_ln_affine:
                layernorm(h16, hf_t,
                          [lnfg[k * 128:(k + 1) * 128, :] for k in range(NKT)],
                          [lnfb[k * 128:(k + 1) * 128, :] for k in range(NKT)])
            else:
                layernorm(h16, hf_t, None, None)
            cps = psA.tile([NCLS * PP * PP, S], F32, tag="A", name="clsps")
            for kt in range(NKT):
                cwt = wp.tile([128, NCLS * PP * PP], BF16, tag="wc")
                nc.sync.dma_start(cwt[:], cls_w[kt * 128:(kt + 1) * 128, :])
                for hf in range(2):
                    nc.tensor.matmul(cps[:, hf * 512:(hf + 1) * 512], cwt[:],
                                     hf_t[kt][:, hf * 512:(hf + 1) * 512],
                                     start=(kt == 0), stop=(kt == NKT - 1),
                                     skip_group_check=True)
            outt = io.tile([NCLS * PP * PP, S], F32, tag="io")
            if use_biases:
                nc.scalar.activation(outt[:], cps[:], AF.Identity,
                                     bias=pcol(clsb[:])[:])
            else:
                nc.scalar.copy(outt[:], cps[:])
            nc.sync.dma_start(out_pl[:], outt[:])


def _prep_host(inputs):
    f = lambda a: np.ascontiguousarray(np.asarray(a), dtype=np.float32)
    import ml_dtypes
    bf = lambda a: np.ascontiguousarray(a).astype(ml_dtypes.bfloat16)
    x = f(inputs["x"])
    conv_w = f(inputs["conv_w"])
    pos = f(inputs["pos_embed"])
    rpb = f(inputs["rpb"])

    xs = []
    for b in range(B):
        xb = x[b].reshape(C_IN, IMG // PP, PP, IMG // PP, PP)
        xb = xb.transpose(0, 2, 4, 1, 3).reshape(KIN, S)
        xp = np.zeros((KIN_PAD, S), np.float32)
        xp[:KIN] = xb
        xs.append(xp)

    w = {}
    cw = conv_w.reshape(D, C_IN, PP, PP).transpose(1, 2, 3, 0).reshape(KIN, D)
    cwp = np.zeros((KIN_PAD, D), np.float32)
    cwp[:KIN] = cw
    w["conv_w"] = cwp
    w["pos_t"] = bf(pos.reshape(S, D).T)
    scale = 1.0 / np.sqrt(np.float32(HD))
    w["wq"] = bf(np.transpose(f(inputs["wq"]), (0, 2, 1)) * scale)
    w["wk"] = bf(np.transpose(f(inputs["wk"]), (0, 2, 1)))
    w["wv"] = bf(np.transpose(f(inputs["wv"]), (0, 2, 1)))
    w["wo"] = bf(np.transpose(f(inputs["wo"]), (0, 2, 1)))
    w["w1"] = bf(np.transpose(f(inputs["w1"]), (0, 2, 1)))
    w["w2"] = bf(np.transpose(f(inputs["w2"]), (0, 2, 1)))
    bc = np.zeros((L, NH, 128, 1920), np.float32)
    for l in range(L):
        for hh in range(NH):
            th = np.ascontiguousarray(rpb[l, :, hh])
            bc[l, hh] = np.lib.stride_tricks.as_strided(
                th[127:], shape=(128, 1920), strides=(-4, 4))
    bc = np.exp(bc)
    bcp = bc.reshape(L, NPAIR, 2, 128, 1920).transpose(0, 1, 3, 2, 4)
    w["bcache"] = bf(bcp.reshape(L, NPAIR, 128, 2 * 1920))
    w["ones1"] = np.ones((1, 128), np.float32)
    w["oavgc"] = bf(np.full((128, 1), 1.0 / D, np.float32))
    sel4 = np.zeros((4, 128), np.float32)
    for p in range(128):
        sel4[p // 32, p] = 1.0
    w["sel4"] = bf(sel4)
    w["cls_w"] = bf(f(inputs["cls_w"]).T)
    vinit = np.zeros((128, VW), np.float32)
    for st in range(NST):
        for hh in range(NH):
            vinit[:, st * VSTRIDE + hh * (HD + 1) + HD] = 1.0
    w["vinit"] = bf(vinit)

    use_biases = any(
        np.abs(f(inputs[k])).max() > 0
        for k in ("bq", "bk", "bv", "bo", "b1", "b2", "conv_b", "cls_b"))
    use_ln_affine = not (
        np.allclose(f(inputs["ln1_s"]), 1.0)
        and np.allclose(f(inputs["ln2_s"]), 1.0)
        and np.allclose(f(inputs["lnf_s"]), 1.0)
        and np.abs(f(inputs["ln1_b"])).max() == 0
        and np.abs(f(inputs["ln2_b"])).max() == 0
        and np.abs(f(inputs["lnf_b"])).max() == 0)
    if use_biases:
        w["bq"] = f(inputs["bq"]).reshape(L, D, 1)
        w["bk"] = f(inputs["bk"]).reshape(L, D, 1)
        w["bv"] = np.ascontiguousarray(
            np.broadcast_to(f(inputs["bv"])[:, None, :], (L, 128, D)))
        w["bo"] = f(inputs["bo"]).reshape(L, D, 1)
        w["b1"] = f(inputs["b1"]).reshape(L, DFF, 1)
        w["b2"] = f(inputs["b2"]).reshape(L, D, 1)
        w["convb"] = f(inputs["conv_b"]).reshape(D, 1)
        w["clsb"] = f(inputs["cls_b"]).reshape(NCLS * PP * PP, 1)
    if use_ln_affine:
        w["ln1g"] = f(inputs["ln1_s"]).reshape(L, D, 1)
        w["ln1b"] = f(inputs["ln1_b"]).reshape(L, D, 1)
        w["ln2g"] = f(inputs["ln2_s"]).reshape(L, D, 1)
        w["ln2b"] = f(inputs["ln2_b"]).reshape(L, D, 1)
        w["lnfg"] = f(inputs["lnf_s"]).reshape(D, 1)
        w["lnfb"] = f(inputs["lnf_b"]).reshape(D, 1)
    return w, xs, use_ln_affine, use_biases


_RUN_KWARGS = {}


def kernel(**inputs):
    w, xs, use_ln_affine, use_biases = _prep_host(inputs)
    nc = bacc.Bacc("TRN2")
    _build(nc, use_ln_affine, use_biases)
    nc.finalize()
    in_maps = [dict(w, x_unf=xs[b]) for b in range(B)]
    res = run_bass_kernel_spmd(nc, in_maps, core_ids=list(range(B)),
                               **_RUN_KWARGS)
    kernel.last_result = res
    out = np.empty((B, NCLS, IMG, IMG), np.float32)
    for b in range(B):
        pl = res.results[b]["out_pl"]
        pl = pl.reshape(NCLS, PP, PP, IMG // PP, IMG // PP)
        out[b] = pl.transpose(0, 3, 1, 4, 2).reshape(NCLS, IMG, IMG)
    return out


# revision 24
# speedup vs baseline: 1.2630x; 1.0141x over previous
"""Trainium2 Bass kernel for nn_CRITTransformer (ViT-style dense transformer).

kernel(**inputs) takes FULL inputs as in reference.setup_inputs() and returns
the FULL [8, 6, 128, 128] output. Data-parallel over batch across 8
NeuronCores (1 image per core), weights replicated.

Per-core layout (v3):
  - activations transposed, bf16 residual stream: h16[d=256 (2 tiles), s=1024]
  - attention per head-pair g: scores^T[k, q] per head j in a [128, 1024]
    PSUM tile (row-tiled K=32 matmuls, ring of 3 so the PE stays ahead of
    the ACT exp stream), Exp per head, one fused bias multiply per pair
    (exp(rpb) Toeplitz cache windows), then col-tiled M=64 PV matmuls
    (2 heads concurrent; softmax denominator rides as the 33rd lhsT
    column -> PSUM rows 32/96). Pass 2 of pair g-1 interleaves with
    pass 1 of pair g.
  - per-c z -> reciprocal -> PE broadcast -> oall scale, WO kt-ordered so
    it starts as soon as oall[0] is scaled
  - LayerNorm: stats via ones-column matmuls into [1, S] PSUM rows, row
    math full-S so Ln/Exp appear once per LN (fewer ACT table swaps)
  - PSUM map: A = 3x[128,1024] (scores ring | QK evac | V-proj | WO pair |
                                FFN fps pair | LN stats rows | embed | cls)
              C = 2x[128,512]  (PV accum | zrep | LN reps | FFN gps)
"""

import numpy as np

import concourse.bass as bass
import concourse.mybir as mybir
import concourse.tile as tile
from concourse import bacc
from concourse.bass_utils import run_bass_kernel_spmd

F32R = mybir.dt.float32r
F32 = mybir.dt.float32
BF16 = mybir.dt.bfloat16
AF = mybir.ActivationFunctionType
OP = mybir.AluOpType

B, C_IN, IMG, PP, D, NH, L, DFF, NCLS, MAXS = 8, 42, 128, 4, 256, 8, 4, 1024, 6, 1024
S = (IMG // PP) ** 2   # 1024
HD = D // NH           # 32
KIN = C_IN * PP * PP   # 672
KIN_PAD = 768
NKT = D // 128         # 2
NST = S // 128         # 8
VSTRIDE = NH * (HD + 1)  # 264 per s-tile in vall
VW = NST * VSTRIDE + 64  # 2176 (pad so M=64 PV lhsT slices stay in-bounds)
EPS = 1e-6
NPAIR = NH // 2        # 4 head pairs


def _build(nc, use_ln_affine, use_biases):
    def din(name, shape, dtype=F32R):
        return nc.dram_tensor(name, shape, dtype, kind="ExternalInput")

    x_unf = din("x_unf", [KIN_PAD, S])
    conv_w = din("conv_w", [KIN_PAD, D])
    pos_t = din("pos_t", [D, S], BF16)
    wq = din("wq", [L, D, D], BF16)
    wk = din("wk", [L, D, D], BF16)
    wv = din("wv", [L, D, D], BF16)
    wo = din("wo", [L, D, D], BF16)
    w1 = din("w1", [L, D, DFF], BF16)
    w2 = din("w2", [L, DFF, D], BF16)
    bcache = din("bcache", [L, NPAIR, 128, 2 * 1920], BF16)
    ones1 = din("ones1", [1, 128], BF16)
    oavgc = din("oavgc", [128, 1], BF16)
    sel4 = din("sel4", [4, 128], BF16)
    vinit = din("vinit", [128, VW], BF16)
    cls_w = din("cls_w", [D, NCLS * PP * PP], BF16)
    if use_biases:
        bq = din("bq", [L, D, 1], F32)
        bk = din("bk", [L, D, 1], F32)
        bv = din("bv", [L, 128, D], F32)
        bo = din("bo", [L, D, 1], F32)
        b1 = din("b1", [L, DFF, 1], F32)
        b2 = din("b2", [L, D, 1], F32)
        convb = din("convb", [D, 1], F32)
        clsb = din("clsb", [NCLS * PP * PP, 1], F32)
    if use_ln_affine:
        ln1g = din("ln1g", [L, D, 1], F32)
        ln1b = din("ln1b", [L, D, 1], F32)
        ln2g = din("ln2g", [L, D, 1], F32)
        ln2b = din("ln2b", [L, D, 1], F32)
        lnfg = din("lnfg", [D, 1], F32)
        lnfb = din("lnfb", [D, 1], F32)

    out_pl = nc.dram_tensor("out_pl", [NCLS * PP * PP, S], F32,
                            kind="ExternalOutput")

    with tile.TileContext(nc) as tc:
        with (
            tc.tile_pool(name="res", bufs=1) as res,
            tc.tile_pool(name="io", bufs=3) as io,
            tc.tile_pool(name="wp", bufs=10) as wp,
            tc.tile_pool(name="w1p", bufs=3) as w1p,
            tc.tile_pool(name="w2p", bufs=9) as w2p,
            tc.tile_pool(name="bcp", bufs=3) as bcp,
            tc.tile_pool(name="ep", bufs=10) as ep,
            tc.tile_pool(name="sgp", bufs=4) as sgp,
            tc.tile_pool(name="rowp", bufs=8) as rowp,
            tc.tile_pool(name="msc", bufs=4) as msc,
            tc.tile_pool(name="gtp", bufs=4) as gtp,
            tc.tile_pool(name="pcl", bufs=4) as pcl,
            tc.tile_pool(name="psA", bufs=3, space="PSUM") as psA,  # 3x4KB
            tc.tile_pool(name="psC", bufs=2, space="PSUM") as psC,  # 2x2KB
        ):
            # ---- constants ----
            ones1_t = res.tile([1, 128], BF16, tag="ones1")
            nc.sync.dma_start(ones1_t[:], ones1[:])
            oavgc_t = res.tile([128, 1], BF16, tag="oavgc")
            nc.sync.dma_start(oavgc_t[:], oavgc[:])
            sel4_t = res.tile([4, 128], BF16, tag="sel4")
            nc.sync.dma_start(sel4_t[:], sel4[:])
            epst = res.tile([128, 1], F32, tag="eps")
            nc.vector.memset(epst[:], EPS)

            h16 = [res.tile([128, S], BF16, tag=f"h16{k}", name=f"h16_{k}")
                   for k in range(NKT)]
            xr = [res.tile([128, S], BF16, tag=f"xr{k}", name=f"xr{k}")
                  for k in range(NKT)]
            qt = [res.tile([128, S], BF16, tag=f"qt{c}", name=f"qt{c}")
                  for c in range(NKT)]
            ktsb = [res.tile([128, S], BF16, tag=f"kt{c}", name=f"ktsb{c}")
                    for c in range(NKT)]
            oall = [res.tile([128, S], BF16, tag=f"oall{c}", name=f"oall{c}")
                    for c in range(NKT)]
            vall = res.tile([128, VW], BF16, tag="vall")
            zall = res.tile([36, S], BF16, tag="zall")
            hf_t = [res.tile([128, S], BF16, tag=f"hf{i}", name=f"hf{i}")
                    for i in range(NKT)]
            nc.sync.dma_start(vall[:], vinit[:])

            def pcol(src_ap):
                t = pcl.tile([128, 1], F32, tag="pcol", name="pcol")
                nc.sync.dma_start(t[:src_ap.shape[0], :], src_ap)
                return t

            # ---- PE warm-up spin: keep HAM at K=8/8 from the start ----
            wub = res.tile([128, 512], BF16, tag="wub")
            nc.vector.memset(wub[:], 0.0)
            wups = psC.tile([128, 512], F32, tag="C", name="wups")
            for _ in range(48):
                nc.tensor.matmul(wups[:], wub[:, 0:128], wub[:],
                                 start=True, stop=True, skip_group_check=True)

            # ================= patch embedding =================
            for c in range(NKT):
                cps = psA.tile([128, S], F32, tag="A", name="emb")
                for kt in range(6):
                    xt_ = io.tile([128, S], F32R, tag="io")
                    nc.sync.dma_start(xt_[:], x_unf[kt * 128:(kt + 1) * 128, :])
                    wt = wp.tile([128, 128], F32R, tag="wc")
                    nc.sync.dma_start(
                        wt[:], conv_w[kt * 128:(kt + 1) * 128,
                                      c * 128:(c + 1) * 128])
                    for hf in range(2):
                        nc.tensor.matmul(
                            cps[:, hf * 512:(hf + 1) * 512],
                            wt[:], xt_[:, hf * 512:(hf + 1) * 512],
                            start=(kt == 0), stop=(kt == 5),
                            skip_group_check=True)
                post = io.tile([128, S], BF16, tag="io")
                nc.sync.dma_start(post[:], pos_t[c * 128:(c + 1) * 128, :])
                if use_biases:
                    u = msc.tile([128, S], F32, tag="sq", name="embu")
                    nc.vector.tensor_tensor(u[:], cps[:], post[:], OP.add)
                    nc.scalar.activation(h16[c][:], u[:], AF.Identity,
                                         bias=pcol(convb[c * 128:(c + 1) * 128, :])[:])
                else:
                    nc.vector.tensor_tensor(h16[c][:], cps[:], post[:], OP.add)

            # ================= layernorm helper =================
            def layernorm(xt, out_t, g_ap, b_ap):
                # squares (bf16, 2x rate)
                sq = []
                for kt in range(NKT):
                    t = msc.tile([128, S], BF16, tag="sq", name=f"lnsq{kt}")
                    nc.vector.tensor_tensor(t[:], xt[kt][:], xt[kt][:], OP.mult)
                    sq.append(t)
                mrow = psA.tile([1, S], F32, tag="A", name="mrow")
                qrow = psA.tile([1, S], F32, tag="A", name="qrow")
                for qh in range(2):
                    qsl = slice(qh * 512, qh * 512 + 512)
                    for kt in range(NKT):
                        nc.tensor.matmul(mrow[:, qsl], oavgc_t[:],
                                         xt[kt][:, qsl],
                                         start=(kt == 0), stop=(kt == NKT - 1),
                                         skip_group_check=True)
                        nc.tensor.matmul(qrow[:, qsl], oavgc_t[:],
                                         sq[kt][:, qsl],
                                         start=(kt == 0), stop=(kt == NKT - 1),
                                         skip_group_check=True)
                m2 = rowp.tile([1, S], F32, tag="row", name="m2")
                nc.scalar.activation(m2[:], mrow[:], AF.Square)
                mr = rowp.tile([1, S], F32, tag="row", name="mr")
                nc.vector.tensor_copy(mr[:], mrow[:])
                var = rowp.tile([1, S], F32, tag="row", name="var")
                nc.vector.tensor_tensor(var[:], qrow[:], m2[:], OP.subtract)
                lnv = rowp.tile([1, S], F32, tag="row", name="lnv")
                nc.scalar.activation(lnv[:], var[:], AF.Ln, bias=epst[0:1, :])
                rrow = rowp.tile([1, S], BF16, tag="row", name="rrow")
                nc.scalar.activation(rrow[:], lnv[:], AF.Exp, scale=-0.5)
                arow = rowp.tile([1, S], BF16, tag="row", name="arow")
                nc.vector.scalar_tensor_tensor(arow[:], mr[:], -1.0,
                                               rrow[:], OP.mult, OP.mult)
                for qh in range(2):
                    qsl = slice(qh * 512, qh * 512 + 512)
                    rrep = psC.tile([128, 512], F32, tag="C", name="rrep")
                    nc.tensor.matmul(rrep[:], ones1_t[:], rrow[:, qsl],
                                     start=True, stop=True,
                                     skip_group_check=True)
                    arep = psC.tile([128, 512], F32, tag="C", name="arep")
                    nc.tensor.matmul(arep[:], ones1_t[:], arow[:, qsl],
                                     start=True, stop=True,
                                     skip_group_check=True)
                    for kt in range(NKT):
                        u = msc.tile([128, 512], F32, tag="sq", name="lnu")
                        nc.vector.tensor_tensor(u[:], xt[kt][:, qsl], rrep[:],
                                                OP.mult)
                        if g_ap is None:
                            nc.vector.tensor_tensor(out_t[kt][:, qsl], u[:],
                                                    arep[:], OP.add)
                        else:
                            u2 = msc.tile([128, 512], F32, tag="sq", name="lnu2")
                            nc.vector.tensor_tensor(u2[:], u[:], arep[:], OP.add)
                            nc.scalar.activation(out_t[kt][:, qsl], u2[:],
                                                 AF.Identity,
                                                 scale=pcol(g_ap[kt])[:],
                                                 bias=pcol(b_ap[kt])[:])

            # ================= transformer layers =================
            for l in range(L):
                # ---- Q^T, K^T ----
                qk_w = []
                for c in range(NKT):
                    wqt = wp.tile([128, D], BF16, tag="wqk", name=f"wq{c}")
                    wkt = wp.tile([128, D], BF16, tag="wqk", name=f"wk{c}")
                    for kt in range(NKT):
                        nc.sync.dma_start(
                            wqt[:, kt * 128:(kt + 1) * 128],
                            wq[l, kt * 128:(kt + 1) * 128,
                               c * 128:(c + 1) * 128])
                        nc.sync.dma_start(
                            wkt[:, kt * 128:(kt + 1) * 128],
                            wk[l, kt * 128:(kt + 1) * 128,
                               c * 128:(c + 1) * 128])
                    qk_w.append((wqt, wkt))

                wo_t = {}
                for c in range(NKT):
                    for kt in range(NKT):
                        wot = wp.tile([128, 128], BF16, tag="wc")
                        nc.sync.dma_start(
                            wot[:], wo[l, kt * 128:(kt + 1) * 128,
                                       c * 128:(c + 1) * 128])
                        wo_t[(c, kt)] = wot
                w1t = [w1p.tile([128, DFF], BF16, tag="w1", name=f"w1t{i}")
                       for i in range(NKT)]
                for kt in range(NKT):
                    nc.sync.dma_start(w1t[kt][:],
                                      w1[l, kt * 128:(kt + 1) * 128, :])
                w2t = [w2p.tile([128, D], BF16, tag="w2", name=f"w2t{i}")
                       for i in range(DFF // 128)]
                for kt in range(DFF // 128):
                    nc.sync.dma_start(w2t[kt][:],
                                      w2[l, kt * 128:(kt + 1) * 128, :])

                def emit_qk(c):
                    wqt, wkt = qk_w[c]
                    qps = psA.tile([128, S], F32, tag="A", name="qps")
                    kps = psA.tile([128, S], F32, tag="A", name="kps")
                    for qh in range(2):
                        qsl = slice(qh * 512, qh * 512 + 512)
                        for kt in range(NKT):
                            nc.tensor.matmul(qps[:, qsl],
                                             wqt[:, kt * 128:(kt + 1) * 128],
                                             h16[kt][:, qsl], start=(kt == 0),
                                             stop=(kt == NKT - 1),
                                             skip_group_check=True)
                            nc.tensor.matmul(kps[:, qsl],
                                             wkt[:, kt * 128:(kt + 1) * 128],
                                             h16[kt][:, qsl], start=(kt == 0),
                                             stop=(kt == NKT - 1),
                                             skip_group_check=True)
                    if use_biases:
                        nc.scalar.activation(
                            qt[c][:], qps[:], AF.Identity,
                            bias=pcol(bq[l, c * 128:(c + 1) * 128, :])[:])
                        nc.scalar.activation(
                            ktsb[c][:], kps[:], AF.Identity,
                            bias=pcol(bk[l, c * 128:(c + 1) * 128, :])[:])
                    else:
                        nc.scalar.copy(qt[c][:], qps[:])
                        nc.scalar.copy(ktsb[c][:], kps[:])

                emit_qk(0)

                # ---- V (s-partition layout, interleaved ones cols) ----
                wvt = [w1p.tile([128, D], BF16, tag="wv", name=f"wv{i}")
                       for i in range(NKT)]
                for kt in range(NKT):
                    nc.sync.dma_start(wvt[kt][:],
                                      wv[l, kt * 128:(kt + 1) * 128, :])
                if use_biases:
                    bvt = msc.tile([128, D], F32, tag="sq", name="bvt")
                    nc.sync.dma_start(bvt[:], bv[l])
                def emit_v(st):
                    vps = psA.tile([128, D], F32, tag="A", name="vps")
                    for kt in range(NKT):
                        nc.tensor.matmul(
                            vps[:], h16[kt][:, st * 128:(st + 1) * 128],
                            wvt[kt][:], start=(kt == 0),
                            stop=(kt == NKT - 1), skip_group_check=True)
                    base = st * VSTRIDE
                    dst = bass.AP(vall.tensor, vall[:].offset + base,
                                  [list(vall[:].ap[0]), [HD + 1, NH], [1, HD]])
                    if use_biases:
                        nc.vector.tensor_tensor(
                            dst, vps[:].rearrange("p (a b) -> p a b", a=NH),
                            bvt[:].rearrange("p (a b) -> p a b", a=NH), OP.add)
                    else:
                        nc.vector.tensor_copy(
                            dst, vps[:].rearrange("p (a b) -> p a b", a=NH))
                emit_qk(1)

                # ---- attention: software-pipelined pairs ----
                et_tiles = {}
                pv_tiles = {}
                bc_tiles = {}
                for g in range(NPAIR):
                    t = bcp.tile([128, 2 * 1920], BF16, tag="bc",
                                 name=f"bct{g}")
                    nc.sync.dma_start(t[:], bcache[l, g])
                    bc_tiles[g] = t

                def emit_pass1_kt(g, kt8):
                    chunk = g // 2
                    r0 = 64 * (g % 2)
                    et = ep.tile([128, 2048], BF16, tag="e", name=f"et{g}_{kt8}")
                    scts = [psA.tile([128, S], F32, tag="A", name="sct")
                            for _ in range(2)]
                    for j in range(2):
                        r = r0 + 32 * j
                        for qh in range(2):
                            nc.tensor.matmul(
                                scts[j][:, qh * 512:qh * 512 + 512],
                                ktsb[chunk][r:r + 32,
                                            kt8 * 128:(kt8 + 1) * 128],
                                qt[chunk][r:r + 32,
                                          qh * 512:(qh + 1) * 512],
                                start=True, stop=True,
                                skip_group_check=True,
                                tile_position=(r, 0))
                    for j in range(2):
                        nc.scalar.activation(et[:, j * 1024:j * 1024 + 1024],
                                             scts[j][:], AF.Exp)
                    bct = bc_tiles[g]
                    bsrc = bass.AP(bct.tensor,
                                   bct[:].offset + (7 - kt8) * 128,
                                   [list(bct[:].ap[0]), [1920, 2], [1, 1024]])
                    nc.vector.tensor_tensor(
                        et[:].rearrange("p (a b) -> p a b", a=2),
                        et[:].rearrange("p (a b) -> p a b", a=2),
                        bsrc, OP.mult)
                    et_tiles[(g, kt8)] = et

                def emit_pass2_kt(g, kt8):
                    et = et_tiles.pop((g, kt8))
                    pv = pv_tiles[g]
                    base = kt8 * VSTRIDE
                    for qh in range(2):
                        for j in range(2):
                            h = 2 * g + j
                            nc.tensor.matmul(
                                pv[qh][64 * j:64 * j + 64, :],
                                vall[:, base + h * (HD + 1):
                                     base + h * (HD + 1) + 64],
                                et[:, j * 1024 + qh * 512:
                                   j * 1024 + qh * 512 + 512],
                                start=(kt8 == 0), stop=(kt8 == NST - 1),
                                skip_group_check=True,
                                tile_position=(0, 64 * j))

                def emit_pair_tail(g):
                    pv = pv_tiles.pop(g)
                    for qh in range(2):
                        stg = sgp.tile([128, 512], BF16, tag="stage",
                                       name="stg")
                        nc.vector.tensor_copy(stg[:], pv[qh][:])
                        c = g // 2
                        r = 64 * (g % 2)
                        qsl = slice(qh * 512, qh * 512 + 512)
                        nc.sync.dma_start(oall[c][r:r + 32, qsl], stg[0:32, :])
                        nc.sync.dma_start(oall[c][r + 32:r + 64, qsl],
                                          stg[64:96, :])
                        zr0 = 32 * (g // 2) + 2 * (g % 2)
                        nc.sync.dma_start(zall[zr0:zr0 + 1, qsl],
                                          stg[32:33, :])
                        nc.sync.dma_start(zall[zr0 + 1:zr0 + 2, qsl],
                                          stg[96:97, :])

                def emit_zscale(c):
                    zf = rowp.tile([4, S], F32, tag="row", name="zf")
                    nc.vector.tensor_copy(zf[:], zall[32 * c:32 * c + 4, :])
                    zr = rowp.tile([4, S], F32, tag="row", name="zr")
                    nc.vector.reciprocal_approx_fast(zr[:], zf[:])
                    zrc = rowp.tile([4, S], BF16, tag="row", name="zrc")
                    nc.vector.tensor_copy(zrc[:], zr[:])
                    for qh in range(2):
                        qsl = slice(qh * 512, qh * 512 + 512)
                        zrep = psC.tile([128, 512], F32, tag="C", name="zrep")
                        nc.tensor.matmul(zrep[:], sel4_t[:], zrc[:, qsl],
                                         start=True, stop=True,
                                         skip_group_check=True)
                        nc.vector.tensor_tensor(oall[c][:, qsl],
                                                oall[c][:, qsl], zrep[:],
                                                OP.mult)

                def alloc_pv(g):
                    pv_tiles[g] = [psC.tile([128, 512], F32, tag="C",
                                            name=f"pv{g}_{qh}")
                                   for qh in range(2)]

                for g in range(NPAIR):
                    for kt8 in range(NST):
                        if g > 0:
                            if kt8 == 0:
                                alloc_pv(g - 1)
                            emit_pass2_kt(g - 1, kt8)
                        else:
                            emit_v(kt8)
                        emit_pass1_kt(g, kt8)
                    if g >= 1:
                        emit_pair_tail(g - 1)
                    if g == 2:
                        emit_zscale(0)
                alloc_pv(NPAIR - 1)
                for kt8 in range(NST):
                    emit_pass2_kt(NPAIR - 1, kt8)
                emit_pair_tail(NPAIR - 1)

                # ---- z-normalize + wo + residual ----
                aps = [psA.tile([128, S], F32, tag="A", name=f"wops{c}")
                       for c in range(NKT)]
                for kt in range(NKT):
                    if kt == 1:
                        emit_zscale(1)
                    for c in range(NKT):
                        for hf in range(2):
                            nc.tensor.matmul(
                                aps[c][:, hf * 512:(hf + 1) * 512],
                                wo_t[(c, kt)][:],
                                oall[kt][:, hf * 512:(hf + 1) * 512],
                                start=(kt == 0), stop=(kt == NKT - 1),
                                skip_group_check=True)
                for c in range(NKT):
                    if use_biases:
                        nc.vector.scalar_tensor_tensor(
                            xr[c][:], aps[c][:],
                            pcol(bo[l, c * 128:(c + 1) * 128, :])[:],
                            h16[c][:], OP.add, OP.add)
                    else:
                        nc.vector.tensor_tensor(xr[c][:], aps[c][:],
                                                h16[c][:], OP.add)
                if use_ln_affine:
                    layernorm(xr, h16,
                              [ln1g[l, k * 128:(k + 1) * 128, :] for k in range(NKT)],
                              [ln1b[l, k * 128:(k + 1) * 128, :] for k in range(NKT)])
                else:
                    layernorm(xr, h16, None, None)

                # ---- FFN ----
                fps = [psA.tile([128, S], F32, tag="A", name=f"fps{c}")
                       for c in range(NKT)]
                for ch in range(DFF // 128):
                    b1c = (pcol(b1[l, ch * 128:(ch + 1) * 128, :])
                           if use_biases else None)
                    for qh in range(2):
                        qsl = slice(qh * 512, qh * 512 + 512)
                        gps = psC.tile([128, 512], F32, tag="C", name="gps")
                        for kt in range(NKT):
                            nc.tensor.matmul(
                                gps[:], w1t[kt][:, ch * 128:(ch + 1) * 128],
                                h16[kt][:, qsl], start=(kt == 0),
                                stop=(kt == NKT - 1), skip_group_check=True)
                        gt = gtp.tile([128, 512], BF16, tag="gt", name="gt")
                        if qh == 0:
                            nc.scalar.activation(
                                gt[:], gps[:], AF.Relu,
                                bias=(b1c[:] if b1c is not None else 0.0))
                        elif b1c is not None:
                            nc.vector.tensor_scalar(
                                gt[:], gps[:], b1c[:], 0.0, OP.add, OP.max)
                        else:
                            nc.vector.tensor_scalar_max(gt[:], gps[:], 0.0)
                        for c in range(NKT):
                            nc.tensor.matmul(
                                fps[c][:, qsl],
                                w2t[ch][:, c * 128:(c + 1) * 128],
                                gt[:], start=(ch == 0),
                                stop=(ch == DFF // 128 - 1),
                                skip_group_check=True)
                for c in range(NKT):
                    if use_biases:
                        nc.vector.scalar_tensor_tensor(
                            xr[c][:], fps[c][:],
                            pcol(b2[l, c * 128:(c + 1) * 128, :])[:],
                            h16[c][:], OP.add, OP.add)
                    else:
                        nc.vector.tensor_tensor(xr[c][:], fps[c][:],
                                                h16[c][:], OP.add)
                if use_ln_affine:
                    layernorm(xr, h16,
                              [ln2g[l, k * 128:(k + 1) * 128, :] for k in range(NKT)],
                              [ln2b[l, k * 128:(k + 1) * 128, :] for k in range(NKT)])
                else:
                    layernorm(xr, h16, None, None)

            # ================= final LN + classifier =================
            if use_ln_affine:
                layernorm(h16, hf_t,
                          [lnfg[k * 128:(k + 1) * 128, :] for k in range(NKT)],
                          [lnfb[k * 128:(k + 1) * 128, :] for k in range(NKT)])
            else:
                layernorm(h16, hf_t, None, None)
            cps = psA.tile([NCLS * PP * PP, S], F32, tag="A", name="clsps")
            for kt in range(NKT):
                cwt = wp.tile([128, NCLS * PP * PP], BF16, tag="wc")
                nc.sync.dma_start(cwt[:], cls_w[kt * 128:(kt + 1) * 128, :])
                for hf in range(2):
                    nc.tensor.matmul(cps[:, hf * 512:(hf + 1) * 512], cwt[:],
                                     hf_t[kt][:, hf * 512:(hf + 1) * 512],
                                     start=(kt == 0), stop=(kt == NKT - 1),
                                     skip_group_check=True)
            outt = io.tile([NCLS * PP * PP, S], F32, tag="io")
            if use_biases:
                nc.scalar.activation(outt[:], cps[:], AF.Identity,
                                     bias=pcol(clsb[:])[:])
            else:
                nc.scalar.copy(outt[:], cps[:])
            nc.sync.dma_start(out_pl[:], outt[:])


def _prep_host(inputs):
    f = lambda a: np.ascontiguousarray(np.asarray(a), dtype=np.float32)
    import ml_dtypes
    bf = lambda a: np.ascontiguousarray(a).astype(ml_dtypes.bfloat16)
    x = f(inputs["x"])
    conv_w = f(inputs["conv_w"])
    pos = f(inputs["pos_embed"])
    rpb = f(inputs["rpb"])

    xs = []
    for b in range(B):
        xb = x[b].reshape(C_IN, IMG // PP, PP, IMG // PP, PP)
        xb = xb.transpose(0, 2, 4, 1, 3).reshape(KIN, S)
        xp = np.zeros((KIN_PAD, S), np.float32)
        xp[:KIN] = xb
        xs.append(xp)

    w = {}
    cw = conv_w.reshape(D, C_IN, PP, PP).transpose(1, 2, 3, 0).reshape(KIN, D)
    cwp = np.zeros((KIN_PAD, D), np.float32)
    cwp[:KIN] = cw
    w["conv_w"] = cwp
    w["pos_t"] = bf(pos.reshape(S, D).T)
    scale = 1.0 / np.sqrt(np.float32(HD))
    w["wq"] = bf(np.transpose(f(inputs["wq"]), (0, 2, 1)) * scale)
    w["wk"] = bf(np.transpose(f(inputs["wk"]), (0, 2, 1)))
    w["wv"] = bf(np.transpose(f(inputs["wv"]), (0, 2, 1)))
    w["wo"] = bf(np.transpose(f(inputs["wo"]), (0, 2, 1)))
    w["w1"] = bf(np.transpose(f(inputs["w1"]), (0, 2, 1)))
    w["w2"] = bf(np.transpose(f(inputs["w2"]), (0, 2, 1)))
    bc = np.zeros((L, NH, 128, 1920), np.float32)
    for l in range(L):
        for hh in range(NH):
            th = np.ascontiguousarray(rpb[l, :, hh])
            bc[l, hh] = np.lib.stride_tricks.as_strided(
                th[127:], shape=(128, 1920), strides=(-4, 4))
    bc = np.exp(bc)
    bcp = bc.reshape(L, NPAIR, 2, 128, 1920).transpose(0, 1, 3, 2, 4)
    w["bcache"] = bf(bcp.reshape(L, NPAIR, 128, 2 * 1920))
    w["ones1"] = bf(np.ones((1, 128), np.float32))
    w["oavgc"] = bf(np.full((128, 1), 1.0 / D, np.float32))
    sel4 = np.zeros((4, 128), np.float32)
    for p in range(128):
        sel4[p // 32, p] = 1.0
    w["sel4"] = bf(sel4)
    w["cls_w"] = bf(f(inputs["cls_w"]).T)
    vinit = np.zeros((128, VW), np.float32)
    for st in range(NST):
        for hh in range(NH):
            vinit[:, st * VSTRIDE + hh * (HD + 1) + HD] = 1.0
    w["vinit"] = bf(vinit)

    use_biases = any(
        np.abs(f(inputs[k])).max() > 0
        for k in ("bq", "bk", "bv", "bo", "b1", "b2", "conv_b", "cls_b"))
    use_ln_affine = not (
        np.allclose(f(inputs["ln1_s"]), 1.0)
        and np.allclose(f(inputs["ln2_s"]), 1.0)
        and np.allclose(f(inputs["lnf_s"]), 1.0)
        and np.abs(f(inputs["ln1_b"])).max() == 0
        and np.abs(f(inputs["ln2_b"])).max() == 0
        and np.abs(f(inputs["lnf_b"])).max() == 0)
    if use_biases:
        w["bq"] = f(inputs["bq"]).reshape(L, D, 1)
        w["bk"] = f(inputs["bk"]).reshape(L, D, 1)
        w["bv"] = np.ascontiguousarray(
            np.broadcast_to(f(inputs["bv"])[:, None, :], (L, 128, D)))
        w["bo"] = f(inputs["bo"]).reshape(L, D, 1)
        w["b1"] = f(inputs["b1"]).reshape(L, DFF, 1)
        w["b2"] = f(inputs["b2"]).reshape(L, D, 1)
        w["convb"] = f(inputs["conv_b"]).reshape(D, 1)
        w["clsb"] = f(inputs["cls_b"]).reshape(NCLS * PP * PP, 1)
    if use_ln_affine:
        w["ln1g"] = f(inputs["ln1_s"]).reshape(L, D, 1)
        w["ln1b"] = f(inputs["ln1_b"]).reshape(L, D, 1)
        w["ln2g"] = f(inputs["ln2_s"]).reshape(L, D, 1)
        w["ln2b"] = f(inputs["ln2_b"]).reshape(L, D, 1)
        w["lnfg"] = f(inputs["lnf_s"]).reshape(D, 1)
        w["lnfb"] = f(inputs["lnf_b"]).reshape(D, 1)
    return w, xs, use_ln_affine, use_biases


_RUN_KWARGS = {}


def kernel(**inputs):
    w, xs, use_ln_affine, use_biases = _prep_host(inputs)
    nc = bacc.Bacc("TRN2")
    _build(nc, use_ln_affine, use_biases)
    nc.finalize()
    in_maps = [dict(w, x_unf=xs[b]) for b in range(B)]
    res = run_bass_kernel_spmd(nc, in_maps, core_ids=list(range(B)),
                               **_RUN_KWARGS)
    kernel.last_result = res
    out = np.empty((B, NCLS, IMG, IMG), np.float32)
    for b in range(B):
        pl = res.results[b]["out_pl"]
        pl = pl.reshape(NCLS, PP, PP, IMG // PP, IMG // PP)
        out[b] = pl.transpose(0, 3, 1, 4, 2).reshape(NCLS, IMG, IMG)
    return out
